# revision 1
# baseline (speedup 1.0000x reference)
"""Trainium2 Bass kernel for LMPNN-style GNN message passing + entity double-matmul.

Reference computation:
    msg      = (x[src] + rel_emb[rel]) * (1 - 2*neg)        # [E, D]
    aggr_out = segment_sum(msg, dst, N)                     # [N, D]
    aggr     = 0.1*x + aggr_out
    score    = relu((aggr @ E^T) * scale + bias)            # [N, V]
    out      = score @ E                                    # [N, D]

Strategy (8 NeuronCores, node-sharded, no collectives):
  * Core c owns nodes [c*512, (c+1)*512).
  * Message passing is re-expressed densely:  aggr = A @ x + R @ rel_emb,
    where A[n, m] = sum of (1-2*neg) over edges m->n  (+0.1 on the diagonal
    for the residual term) and R[n, r] = sum of (1-2*neg) over edges with
    relation r landing on n. The host builds the integer-valued A/R count
    matrices from the index tensors (pure index preprocessing); the device
    does all floating-point work as dense TensorEngine matmuls accumulated
    in fp32 PSUM, producing aggrT [D, 512] directly.
  * The double matmul streams the (host-transposed / host-swizzled) entity
    table from HBM in bf16, interleaving per-128-entity chunks:
    scoreT = ET_chunk(lhsT) x aggrT -> relu(+scale/bias) on ACT/DVE ->
    outT += E_chunk(lhsT) x scoreT accumulated in a single PSUM bank.
  * Output is outT [128, 512] fp32 per core; host transposes/concats.
"""

import sys

import numpy as np

try:
    import concourse.bass as bass
except ImportError:  # pragma: no cover
    sys.path.insert(0, "/opt/trn_rl_repo")
    import concourse.bass as bass

import ml_dtypes

import concourse.bacc as bacc
import concourse.mybir as mybir
import concourse.tile as tile
from concourse.bass_utils import run_bass_kernel_spmd

BF16 = ml_dtypes.bfloat16
F32 = np.float32


class Cfg:
    def __init__(self, N=4096, E=262144, D=128, R=1000, V=50000, C=8):
        self.N, self.E, self.D, self.R, self.V, self.C = N, E, D, R, V, C
        self.NPC = N // C                       # nodes per core
        assert self.NPC % 128 == 0 and N % 128 == 0
        self.RPAD = ((R + 127) // 128) * 128    # padded relation count
        self.VPAD = ((V + 511) // 512) * 512    # padded entity count
        self.NV = self.VPAD // 128              # 128-entity chunks
        self.NKX = N // 128                     # k-chunks for A @ x
        self.NKR = self.RPAD // 128             # k-chunks for R @ rel


def host_prep(cfg, x, edge_index, relation_id, neg_flag, rel_emb, entity_emb,
              scale, bias):
    """Build per-core in_maps. The host only converts the edge/index tensors
    into dense count matrices + does layout/dtype conversion; all FP math on
    the embeddings happens on device."""
    src = np.asarray(edge_index[0]).astype(np.int64)
    dst = np.asarray(edge_index[1]).astype(np.int64)
    rel = np.asarray(relation_id).astype(np.int64)
    neg = np.asarray(neg_flag).astype(np.int64)
    x = np.asarray(x, F32)
    rel_emb = np.asarray(rel_emb, F32)
    entity_emb = np.asarray(entity_emb, F32)
    scale = np.asarray(scale, F32)
    bias = np.asarray(bias, F32)

    C, NPC, D = cfg.C, cfg.NPC, cfg.D
    negc = (1.0 - 2.0 * neg).astype(F32)

    # dense message-passing operators (index preprocessing)
    A = np.zeros((cfg.N, cfg.N), F32)
    np.add.at(A, (dst, src), negc)
    A[np.arange(cfg.N), np.arange(cfg.N)] += 0.1          # residual 0.1*x
    Rm = np.zeros((cfg.N, cfg.RPAD), F32)
    np.add.at(Rm, (dst, rel), negc)

    # shared (replicated) tensors
    vpad = cfg.VPAD
    E_pad = np.zeros((vpad, D), F32)
    E_pad[: cfg.V] = entity_emb
    et_tab = np.ascontiguousarray(E_pad.T).astype(BF16)            # [128, VPAD]
    e_sw = np.ascontiguousarray(
        E_pad.reshape(vpad // 512, 4, 128, D).transpose(0, 2, 1, 3)
    ).astype(BF16)                                                 # [VPAD/512,128,4,D]
    scale_pad = np.ones(vpad, F32)
    scale_pad[: cfg.V] = scale
    bias_pad = np.zeros(vpad, F32)
    bias_pad[: cfg.V] = bias
    scaleT = np.ascontiguousarray(scale_pad.reshape(cfg.NV, 128).T)
    biasT = np.ascontiguousarray(bias_pad.reshape(cfg.NV, 128).T)
    fast_relu = bool(np.all(scale == 1.0) and np.all(bias == 0.0))

    xb = x.astype(BF16)                                            # [N, D]
    rb = np.zeros((cfg.RPAD, D), F32)
    rb[: cfg.R] = rel_emb
    rb = rb.astype(BF16)

    shared = {
        "x_b": xb, "rel_b": rb, "et_tab": et_tab, "e_sw": e_sw,
        "scaleT": scaleT, "biasT": biasT,
    }
    in_maps = []
    for c in range(C):
        rows = slice(c * NPC, (c + 1) * NPC)
        at_c = np.ascontiguousarray(A[rows].T).astype(BF16)        # [N, NPC]
        rt_c = np.ascontiguousarray(Rm[rows].T).astype(BF16)       # [RPAD, NPC]
        m = dict(shared)
        m.update({"a_t": at_c, "r_t": rt_c})
        in_maps.append(m)
    return in_maps, fast_relu


def build(cfg, fast_relu, enable_asserts=False, dve_mod=2, dve_thresh=1):
    f32, bf16 = mybir.dt.float32, mybir.dt.bfloat16
    nc = bacc.Bacc(
        "TRN2", target_bir_lowering=False, debug=False,
        enable_asserts=enable_asserts,
    )
    D, NPC, NV = cfg.D, cfg.NPC, cfg.NV

    xb_t = nc.dram_tensor("x_b", [cfg.N, D], bf16, kind="ExternalInput").ap()
    rb_t = nc.dram_tensor("rel_b", [cfg.RPAD, D], bf16, kind="ExternalInput").ap()
    at_t = nc.dram_tensor("a_t", [cfg.N, NPC], bf16, kind="ExternalInput").ap()
    rt_t = nc.dram_tensor("r_t", [cfg.RPAD, NPC], bf16, kind="ExternalInput").ap()
    ett_t = nc.dram_tensor("et_tab", [128, cfg.VPAD], bf16, kind="ExternalInput").ap()
    esw_t = nc.dram_tensor("e_sw", [cfg.VPAD // 512, 128, 4, D], bf16, kind="ExternalInput").ap()
    scl_t = nc.dram_tensor("scaleT", [128, NV], f32, kind="ExternalInput").ap()
    bia_t = nc.dram_tensor("biasT", [128, NV], f32, kind="ExternalInput").ap()
    out_t = nc.dram_tensor("out", [128, NPC], f32, kind="ExternalOutput").ap()

    Relu = mybir.ActivationFunctionType.Relu

    with tile.TileContext(nc) as tc:
        with (
            tc.tile_pool(name="const", bufs=1) as constp,
            tc.tile_pool(name="aggk", bufs=4) as akp,
            tc.tile_pool(name="etab", bufs=6) as ep,
            tc.tile_pool(name="scoresb", bufs=6) as scp,
            tc.tile_pool(name="psA", bufs=1, space="PSUM") as psA,
            tc.tile_pool(name="psS", bufs=6, space="PSUM") as psS,
            tc.tile_pool(name="psO", bufs=1, space="PSUM") as psO,
        ):
            sclt = constp.tile([128, NV], f32, tag="sc")
            nc.sync.dma_start(sclt, scl_t)
            biat = constp.tile([128, NV], f32, tag="bi")
            nc.sync.dma_start(biat, bia_t)
            aggrT_sb = constp.tile([128, NPC], bf16, tag="aggrT")
            out_sb = constp.tile([128, NPC], f32, tag="outsb")

            # ---- phase 1: aggrT = x^T A^T + rel^T R^T  (k-chunked) --------
            aggr_ps = psA.tile([128, NPC], f32, tag="aggrps")
            for k in range(cfg.NKX):
                ks = slice(k * 128, (k + 1) * 128)
                xk = akp.tile([128, D], bf16, tag="lhs")
                nc.sync.dma_start(xk, xb_t[ks, :])
                ak = akp.tile([128, NPC], bf16, tag="rhs")
                nc.sync.dma_start(ak, at_t[ks, :])
                nc.tensor.matmul(
                    aggr_ps, lhsT=xk, rhs=ak,
                    start=(k == 0), stop=False, skip_group_check=True,
                )
            for k in range(cfg.NKR):
                ks = slice(k * 128, (k + 1) * 128)
                rk = akp.tile([128, D], bf16, tag="lhs")
                nc.sync.dma_start(rk, rb_t[ks, :])
                rrk = akp.tile([128, NPC], bf16, tag="rhs")
                nc.sync.dma_start(rrk, rt_t[ks, :])
                nc.tensor.matmul(
                    aggr_ps, lhsT=rk, rhs=rrk,
                    start=False, stop=(k == cfg.NKR - 1), skip_group_check=True,
                )
            nc.vector.tensor_copy(aggrT_sb, aggr_ps)

            # ---- phase 2: fused double matmul over entity chunks ----------
            outT_ps = psO.tile([128, NPC], f32, tag="outps")
            for vb in range(cfg.VPAD // 512):
                ett = ep.tile([128, 512], bf16, tag="et")
                nc.sync.dma_start(ett, ett_t[:, vb * 512 : (vb + 1) * 512])
                esw = ep.tile([128, 4, D], bf16, tag="ee")
                nc.sync.dma_start(esw, esw_t[vb])
                for j in range(4):
                    v = vb * 4 + j
                    sps = psS.tile([128, NPC], f32, tag="sps")
                    nc.tensor.matmul(
                        sps, lhsT=ett[:, j * 128 : (j + 1) * 128], rhs=aggrT_sb,
                        start=True, stop=True, skip_group_check=True,
                    )
                    st_sb = scp.tile([128, NPC], bf16, tag="st")
                    if fast_relu:
                        if v % dve_mod < dve_thresh:
                            nc.vector.tensor_relu(st_sb, sps)
                        else:
                            nc.scalar.activation(st_sb, sps, Relu)
                    else:
                        nc.scalar.activation(
                            st_sb, sps, Relu,
                            bias=biat[:, v : v + 1], scale=sclt[:, v : v + 1],
                        )
                    nc.tensor.matmul(
                        outT_ps, lhsT=esw[:, j, :], rhs=st_sb,
                        start=(v == 0), stop=(v == NV - 1), skip_group_check=True,
                    )

            nc.vector.tensor_copy(out_sb, outT_ps)
            nc.sync.dma_start(out_t, out_sb)

    nc.compile()
    return nc


def run(inputs, trace=False, cfg=None, dve_mod=2, dve_thresh=1):
    if cfg is None:
        cfg = Cfg()
    in_maps, fast_relu = host_prep(cfg, **inputs)
    nc = build(cfg, fast_relu, dve_mod=dve_mod, dve_thresh=dve_thresh)
    try:
        res = run_bass_kernel_spmd(
            nc, in_maps, core_ids=list(range(cfg.C)), trace=trace,
        )
    except ModuleNotFoundError:
        # NTFF profiling hook unavailable in this container; run untraced.
        res = run_bass_kernel_spmd(
            nc, in_maps, core_ids=list(range(cfg.C)), trace=False,
        )
    outs = []
    for c in range(cfg.C):
        outs.append(np.ascontiguousarray(np.asarray(res.results[c]["out"]).T))
    full = np.concatenate(outs, axis=0).astype(np.float32)
    return full, res


def kernel(**inputs):
    full, _ = run(inputs, trace=False)
    return full



# revision 3
# speedup vs baseline: 1.0943x; 1.0943x over previous
"""Trainium2 Bass kernel for LMPNN-style GNN message passing + entity double-matmul.

Reference computation:
    msg      = (x[src] + rel_emb[rel]) * (1 - 2*neg)        # [E, D]
    aggr_out = segment_sum(msg, dst, N)                     # [N, D]
    aggr     = 0.1*x + aggr_out
    score    = relu((aggr @ E^T) * scale + bias)            # [N, V]
    out      = score @ E                                    # [N, D]

Strategy (8 NeuronCores, node-sharded, no collectives):
  * Core c owns nodes [c*512, (c+1)*512).
  * Message passing is re-expressed densely:  aggr = A @ x + R @ rel_emb,
    where A[n, m] = sum of (1-2*neg) over edges m->n (+0.1 on the diagonal)
    and R[n, r] the same per relation. Host builds the integer count
    matrices (index preprocessing only); the device does the FP work as
    bf16 TensorEngine matmuls accumulated in fp32 PSUM -> aggrT [D, 512].
  * Fast path (scale==1, bias==0): the double matmul runs in fp8-e4m3 with
    DoubleRow (2x PE throughput) using the identity
        relu(s) = s + relu(-s)
    =>  out = aggr @ G + relu(-s) @ E,   G = E8^T E8 (Gram, on device).
    The dominant coherent term aggr@G is computed exactly (f32r matmul),
    so e4m3 noise only touches the small fluctuation term relu(-s)@E and
    concentrates away over V -> measured rel err ~3.3e-3.
    - MM1: scoresT = (-E8^T dup-slots) x [a_hi|a_lo] pairs  (DoubleRow)
      with aggr/8 split hi/lo in e4m3 to keep the score error incoherent.
    - relu copies PSUM->SBUF-fp8 alternate DVE/ACT (pair-granularity, two
      PSUM banks per op) -- these are the critical resource.
    - MM2: outT += E8-pairs x relu-pairs  (DoubleRow, 256 entities/inst).
    - MMG: G += E8-pairs x E8-pairs       (DoubleRow, on the same stream).
  * General scale/bias path falls back to the bf16 relu kernel.
  * Output is outT [128, 512] fp32 per core; host transposes/concats.
"""

import sys

import numpy as np

try:
    import concourse.bass as bass
except ImportError:  # pragma: no cover
    sys.path.insert(0, "/opt/trn_rl_repo")
    import concourse.bass as bass

import ml_dtypes

import concourse.bacc as bacc
import concourse.mybir as mybir
import concourse.tile as tile
from concourse.bass_utils import run_bass_kernel_spmd

BF16 = ml_dtypes.bfloat16
F8 = ml_dtypes.float8_e4m3
F32 = np.float32

LAST_NC = None


class Cfg:
    def __init__(self, N=4096, E=262144, D=128, R=1000, V=50000, C=8):
        self.N, self.E, self.D, self.R, self.V, self.C = N, E, D, R, V, C
        self.NPC = N // C                       # nodes per core
        assert self.NPC % 128 == 0 and N % 128 == 0
        self.RPAD = ((R + 127) // 128) * 128    # padded relation count
        self.VPAD = ((V + 255) // 256) * 256    # 256-entity super-chunks
        self.NSC = self.VPAD // 256             # super-chunk count (196)
        self.NBLK = (self.VPAD + 2047) // 2048  # DMA blocks (25, last partial)
        self.VPAD2 = self.NBLK * 2048           # esw/etn padded cols (51200)
        self.NKX = N // 128                     # k-chunks for A @ x
        self.NKR = self.RPAD // 128             # k-chunks for R @ rel
        # old-path padding
        self.VPAD_G = ((V + 511) // 512) * 512
        self.NV_G = self.VPAD_G // 128


# --------------------------------------------------------------------------
# fast path: fp8 DoubleRow via relu(s) = s + relu(-s) and the Gram term
# --------------------------------------------------------------------------

def host_prep_fast(cfg, x, edge_index, relation_id, neg_flag, rel_emb,
                   entity_emb):
    src = np.asarray(edge_index[0]).astype(np.int64)
    dst = np.asarray(edge_index[1]).astype(np.int64)
    rel = np.asarray(relation_id).astype(np.int64)
    neg = np.asarray(neg_flag).astype(np.int64)
    x = np.asarray(x, F32)
    rel_emb = np.asarray(rel_emb, F32)
    entity_emb = np.asarray(entity_emb, F32)

    C, NPC, D = cfg.C, cfg.NPC, cfg.D
    negc = (1.0 - 2.0 * neg).astype(F32)

    # dense message-passing operators (index preprocessing)
    A = np.zeros((cfg.N, cfg.N), F32)
    np.add.at(A, (dst, src), negc)
    A[np.arange(cfg.N), np.arange(cfg.N)] += 0.1          # residual 0.1*x
    Rm = np.zeros((cfg.N, cfg.RPAD), F32)
    np.add.at(Rm, (dst, rel), negc)

    # fp8 entity table, padded
    Epad = np.zeros((cfg.VPAD2, D), F32)
    Epad[: cfg.V] = entity_emb
    E8 = Epad.astype(F8)                                   # e4m3 table
    E8f = E8.astype(F32)
    etn = np.ascontiguousarray((-E8f).T).astype(F8)        # [128, VPAD2], negated
    esw = np.ascontiguousarray(
        E8.reshape(cfg.NBLK, 8, 2, 128, D).transpose(0, 3, 1, 2, 4)
    ).reshape(cfg.NBLK, 128, 2048)                         # [25, 128, 2048]

    xb = x.astype(BF16)                                    # [N, D]
    rb = np.zeros((cfg.RPAD, D), F32)
    rb[: cfg.R] = rel_emb
    rb = rb.astype(BF16)

    in_maps = []
    for c in range(C):
        rows = slice(c * NPC, (c + 1) * NPC)
        xa = np.empty((cfg.N, 128 + NPC), BF16)            # [x | A^T] combined
        xa[:, :D] = xb
        xa[:, D:] = np.ascontiguousarray(A[rows].T).astype(BF16)
        ra = np.empty((cfg.RPAD, 128 + NPC), BF16)
        ra[:, :D] = rb
        ra[:, D:] = np.ascontiguousarray(Rm[rows].T).astype(BF16)
        in_maps.append({"xa": xa, "ra": ra, "etn": etn, "esw": esw})
    return in_maps


def build_fast(cfg):
    f32, bf16 = mybir.dt.float32, mybir.dt.bfloat16
    f8, f32r = mybir.dt.float8e4, mybir.dt.float32r
    DR = mybir.MatmulPerfMode.DoubleRow
    Relu = mybir.ActivationFunctionType.Relu
    Copy = mybir.ActivationFunctionType.Copy

    nc = bacc.Bacc("TRN2", target_bir_lowering=False, debug=False,
                   enable_asserts=False)
    D, NPC, NSC, NBLK = cfg.D, cfg.NPC, cfg.NSC, cfg.NBLK

    xa_t = nc.dram_tensor("xa", [cfg.N, 128 + NPC], bf16, kind="ExternalInput").ap()
    ra_t = nc.dram_tensor("ra", [cfg.RPAD, 128 + NPC], bf16, kind="ExternalInput").ap()
    etn_t = nc.dram_tensor("etn", [128, cfg.VPAD2], f8, kind="ExternalInput").ap()
    esw_t = nc.dram_tensor("esw", [NBLK, 128, 2048], f8, kind="ExternalInput").ap()
    out_t = nc.dram_tensor("out", [128, NPC], f32, kind="ExternalOutput").ap()

    xa_r = xa_t.rearrange("(k p) f -> p k f", p=128)       # [128, 32, 640]
    ra_r = ra_t.rearrange("(k p) f -> p k f", p=128)       # [128, 8, 640]

    with tile.TileContext(nc) as tc:
        with (
            tc.tile_pool(name="xap", bufs=4) as xap,
            tc.tile_pool(name="rap", bufs=1) as rap,
            tc.tile_pool(name="prep", bufs=1) as prep,
            tc.tile_pool(name="etnp", bufs=3) as etnp,
            tc.tile_pool(name="eswp", bufs=3) as eswp,
            tc.tile_pool(name="m2p", bufs=3) as m2p,
            tc.tile_pool(name="psA", bufs=1, space="PSUM") as psA,
            tc.tile_pool(name="psG", bufs=1, space="PSUM") as psG,
            tc.tile_pool(name="psS", bufs=2, space="PSUM") as psS,
            tc.tile_pool(name="psO", bufs=1, space="PSUM") as psO,
        ):
            # ---- phase 1: aggrT = x^T A^T + rel^T R^T -------------------
            xat = [xap.tile([128, 8, 128 + NPC], bf16, tag=f"xa{j}",
                            name=f"xa{j}")
                   for j in range(4)]
            for j in range(4):
                nc.sync.dma_start(xat[j], xa_r[:, 8 * j: 8 * j + 8, :])
            rat = rap.tile([128, 8, 128 + NPC], bf16, tag="ra")
            nc.sync.dma_start(rat, ra_r)

            aggr_ps = psA.tile([128, NPC], f32, tag="aggrps")
            for k in range(cfg.NKX):
                t = xat[k // 8]
                nc.tensor.matmul(
                    aggr_ps, lhsT=t[:, k % 8, 0:D], rhs=t[:, k % 8, D:],
                    start=(k == 0), stop=False, skip_group_check=True,
                )
            for k in range(cfg.NKR):
                nc.tensor.matmul(
                    aggr_ps, lhsT=rat[:, k, 0:D], rhs=rat[:, k, D:],
                    start=False, stop=(k == cfg.NKR - 1), skip_group_check=True,
                )

            # ---- one-time prep: aggr/8 in f32 / f32r / fp8 hi+lo --------
            a8f = prep.tile([128, NPC], f32, tag="a8f")
            nc.scalar.activation(a8f, aggr_ps, Copy, scale=0.125)
            a8r = prep.tile([128, NPC], f32r, tag="a8r")
            nc.scalar.activation(a8r, aggr_ps, Copy, scale=0.125)
            a8p = prep.tile([128, 2, NPC], f8, tag="a8p")
            nc.vector.tensor_copy(a8p[:, 0, :], a8f)
            nc.vector.tensor_tensor(a8p[:, 1, :], a8f, a8p[:, 0, :],
                                    mybir.AluOpType.subtract)

            # ---- main loop over entity super-chunks ---------------------
            G_ps = psG.tile([128, 128], f32, tag="gps")
            outT_ps = psO.tile([128, NPC], f32, tag="outps")
            for b in range(NBLK):
                ent = etnp.tile([128, 2048], f8, tag="etn")
                nc.sync.dma_start(ent, etn_t[:, b * 2048:(b + 1) * 2048])
                esb = eswp.tile([128, 8, 2, 128], f8, tag="esw")
                nc.sync.dma_start(esb, esw_t[b])
                n_sc = min(8, NSC - b * 8)
                for s in range(n_sc):
                    g = b * 8 + s
                    # Gram accumulation on the esw pair stream
                    nc.tensor.matmul(
                        G_ps, lhsT=esb[:, s, :, :], rhs=esb[:, s, :, :],
                        start=(g == 0), stop=(g == NSC - 1),
                        perf_mode=DR, skip_group_check=True,
                    )
                    # MM1: scoresT for 2x128 entities (negated weights)
                    sps = psS.tile([128, 1024], f32, tag="sps")
                    for h in range(2):
                        w = ent[:, s * 256 + h * 128: s * 256 + (h + 1) * 128]
                        nc.tensor.matmul(
                            sps[:, h * NPC:(h + 1) * NPC],
                            lhsT=w.unsqueeze(1).broadcast_to([128, 2, 128]),
                            rhs=a8p, start=True, stop=True,
                            perf_mode=DR, skip_group_check=True,
                        )
                    # relu pair-copy PSUM -> SBUF fp8 (the critical resource)
                    m2 = m2p.tile([128, 2, NPC], f8, tag="m2")
                    spsr = sps.rearrange("p (a b) -> p a b", a=2)
                    if g % 11 in (0, 2, 4, 6, 8):
                        nc.vector.tensor_relu(m2, spsr)
                    else:
                        nc.scalar.activation(m2, spsr, Relu)
                    # MM2: outT += E8-pairs x relu-pairs (256 entities)
                    nc.tensor.matmul(
                        outT_ps, lhsT=esb[:, s, :, :], rhs=m2,
                        start=(g == 0), stop=False,
                        perf_mode=DR, skip_group_check=True,
                    )

            # ---- tail: Gram term + final scale --------------------------
            g_sb = prep.tile([128, 128], f32r, tag="gsb")
            nc.scalar.activation(g_sb, G_ps, Copy)
            nc.tensor.matmul(outT_ps, lhsT=g_sb, rhs=a8r,
                             start=False, stop=True, skip_group_check=True)
            fin = prep.tile([128, NPC], f32, tag="fin")
            nc.scalar.activation(fin, outT_ps, Copy, scale=8.0)
            nc.sync.dma_start(out_t, fin)

    nc.compile()
    return nc


# --------------------------------------------------------------------------
# general path (arbitrary scale/bias): bf16 relu kernel (previous baseline)
# --------------------------------------------------------------------------

def host_prep_general(cfg, x, edge_index, relation_id, neg_flag, rel_emb,
                      entity_emb, scale, bias):
    src = np.asarray(edge_index[0]).astype(np.int64)
    dst = np.asarray(edge_index[1]).astype(np.int64)
    rel = np.asarray(relation_id).astype(np.int64)
    neg = np.asarray(neg_flag).astype(np.int64)
    x = np.asarray(x, F32)
    rel_emb = np.asarray(rel_emb, F32)
    entity_emb = np.asarray(entity_emb, F32)
    scale = np.asarray(scale, F32)
    bias = np.asarray(bias, F32)

    C, NPC, D = cfg.C, cfg.NPC, cfg.D
    negc = (1.0 - 2.0 * neg).astype(F32)

    A = np.zeros((cfg.N, cfg.N), F32)
    np.add.at(A, (dst, src), negc)
    A[np.arange(cfg.N), np.arange(cfg.N)] += 0.1
    Rm = np.zeros((cfg.N, cfg.RPAD), F32)
    np.add.at(Rm, (dst, rel), negc)

    vpad = cfg.VPAD_G
    E_pad = np.zeros((vpad, D), F32)
    E_pad[: cfg.V] = entity_emb
    et_tab = np.ascontiguousarray(E_pad.T).astype(BF16)
    e_sw = np.ascontiguousarray(
        E_pad.reshape(vpad // 512, 4, 128, D).transpose(0, 2, 1, 3)
    ).astype(BF16)
    scale_pad = np.ones(vpad, F32)
    scale_pad[: cfg.V] = scale
    bias_pad = np.zeros(vpad, F32)
    bias_pad[: cfg.V] = bias
    scaleT = np.ascontiguousarray(scale_pad.reshape(cfg.NV_G, 128).T)
    biasT = np.ascontiguousarray(bias_pad.reshape(cfg.NV_G, 128).T)

    xb = x.astype(BF16)
    rb = np.zeros((cfg.RPAD, D), F32)
    rb[: cfg.R] = rel_emb
    rb = rb.astype(BF16)

    shared = {"x_b": xb, "rel_b": rb, "et_tab": et_tab, "e_sw": e_sw,
              "scaleT": scaleT, "biasT": biasT}
    in_maps = []
    for c in range(C):
        rows = slice(c * NPC, (c + 1) * NPC)
        m = dict(shared)
        m.update({
            "a_t": np.ascontiguousarray(A[rows].T).astype(BF16),
            "r_t": np.ascontiguousarray(Rm[rows].T).astype(BF16),
        })
        in_maps.append(m)
    return in_maps


def build_general(cfg):
    f32, bf16 = mybir.dt.float32, mybir.dt.bfloat16
    nc = bacc.Bacc("TRN2", target_bir_lowering=False, debug=False,
                   enable_asserts=False)
    D, NPC, NV = cfg.D, cfg.NPC, cfg.NV_G

    xb_t = nc.dram_tensor("x_b", [cfg.N, D], bf16, kind="ExternalInput").ap()
    rb_t = nc.dram_tensor("rel_b", [cfg.RPAD, D], bf16, kind="ExternalInput").ap()
    at_t = nc.dram_tensor("a_t", [cfg.N, NPC], bf16, kind="ExternalInput").ap()
    rt_t = nc.dram_tensor("r_t", [cfg.RPAD, NPC], bf16, kind="ExternalInput").ap()
    ett_t = nc.dram_tensor("et_tab", [128, cfg.VPAD_G], bf16, kind="ExternalInput").ap()
    esw_t = nc.dram_tensor("e_sw", [cfg.VPAD_G // 512, 128, 4, D], bf16, kind="ExternalInput").ap()
    scl_t = nc.dram_tensor("scaleT", [128, NV], f32, kind="ExternalInput").ap()
    bia_t = nc.dram_tensor("biasT", [128, NV], f32, kind="ExternalInput").ap()
    out_t = nc.dram_tensor("out", [128, NPC], f32, kind="ExternalOutput").ap()

    Relu = mybir.ActivationFunctionType.Relu

    with tile.TileContext(nc) as tc:
        with (
            tc.tile_pool(name="const", bufs=1) as constp,
            tc.tile_pool(name="aggk", bufs=4) as akp,
            tc.tile_pool(name="etab", bufs=6) as ep,
            tc.tile_pool(name="scoresb", bufs=6) as scp,
            tc.tile_pool(name="psA", bufs=1, space="PSUM") as psA,
            tc.tile_pool(name="psS", bufs=6, space="PSUM") as psS,
            tc.tile_pool(name="psO", bufs=1, space="PSUM") as psO,
        ):
            sclt = constp.tile([128, NV], f32, tag="sc")
            nc.sync.dma_start(sclt, scl_t)
            biat = constp.tile([128, NV], f32, tag="bi")
            nc.sync.dma_start(biat, bia_t)
            aggrT_sb = constp.tile([128, NPC], bf16, tag="aggrT")
            out_sb = constp.tile([128, NPC], f32, tag="outsb")

            aggr_ps = psA.tile([128, NPC], f32, tag="aggrps")
            for k in range(cfg.NKX):
                ks = slice(k * 128, (k + 1) * 128)
                xk = akp.tile([128, D], bf16, tag="lhs")
                nc.sync.dma_start(xk, xb_t[ks, :])
                ak = akp.tile([128, NPC], bf16, tag="rhs")
                nc.sync.dma_start(ak, at_t[ks, :])
                nc.tensor.matmul(aggr_ps, lhsT=xk, rhs=ak,
                                 start=(k == 0), stop=False, skip_group_check=True)
            for k in range(cfg.NKR):
                ks = slice(k * 128, (k + 1) * 128)
                rk = akp.tile([128, D], bf16, tag="lhs")
                nc.sync.dma_start(rk, rb_t[ks, :])
                rrk = akp.tile([128, NPC], bf16, tag="rhs")
                nc.sync.dma_start(rrk, rt_t[ks, :])
                nc.tensor.matmul(aggr_ps, lhsT=rk, rhs=rrk,
                                 start=False, stop=(k == cfg.NKR - 1),
                                 skip_group_check=True)
            nc.vector.tensor_copy(aggrT_sb, aggr_ps)

            outT_ps = psO.tile([128, NPC], f32, tag="outps")
            for vb in range(cfg.VPAD_G // 512):
                ett = ep.tile([128, 512], bf16, tag="et")
                nc.sync.dma_start(ett, ett_t[:, vb * 512: (vb + 1) * 512])
                esw = ep.tile([128, 4, D], bf16, tag="ee")
                nc.sync.dma_start(esw, esw_t[vb])
                for j in range(4):
                    v = vb * 4 + j
                    sps = psS.tile([128, NPC], f32, tag="sps")
                    nc.tensor.matmul(sps, lhsT=ett[:, j * 128: (j + 1) * 128],
                                     rhs=aggrT_sb, start=True, stop=True,
                                     skip_group_check=True)
                    st_sb = scp.tile([128, NPC], bf16, tag="st")
                    nc.scalar.activation(st_sb, sps, Relu,
                                         bias=biat[:, v: v + 1],
                                         scale=sclt[:, v: v + 1])
                    nc.tensor.matmul(outT_ps, lhsT=esw[:, j, :], rhs=st_sb,
                                     start=(v == 0), stop=(v == NV - 1),
                                     skip_group_check=True)

            nc.vector.tensor_copy(out_sb, outT_ps)
            nc.sync.dma_start(out_t, out_sb)

    nc.compile()
    return nc


# --------------------------------------------------------------------------

def run(inputs, trace=False, cfg=None):
    global LAST_NC
    if cfg is None:
        cfg = Cfg()
    scale = np.asarray(inputs["scale"], F32)
    bias = np.asarray(inputs["bias"], F32)
    fast = bool(np.all(scale == 1.0) and np.all(bias == 0.0))
    if fast:
        in_maps = host_prep_fast(
            cfg, inputs["x"], inputs["edge_index"], inputs["relation_id"],
            inputs["neg_flag"], inputs["rel_emb"], inputs["entity_emb"])
        nc = build_fast(cfg)
    else:
        in_maps = host_prep_general(cfg, **{
            k: inputs[k] for k in
            ["x", "edge_index", "relation_id", "neg_flag", "rel_emb",
             "entity_emb", "scale", "bias"]})
        nc = build_general(cfg)
    LAST_NC = nc
    try:
        res = run_bass_kernel_spmd(
            nc, in_maps, core_ids=list(range(cfg.C)), trace=trace)
    except ModuleNotFoundError:
        res = run_bass_kernel_spmd(
            nc, in_maps, core_ids=list(range(cfg.C)), trace=False)
    outs = []
    for c in range(cfg.C):
        outs.append(np.ascontiguousarray(np.asarray(res.results[c]["out"]).T))
    full = np.concatenate(outs, axis=0).astype(np.float32)
    return full, res


def kernel(**inputs):
    full, _ = run(inputs, trace=False)
    return full


# revision 7
# speedup vs baseline: 1.4676x; 1.3412x over previous
"""Trainium2 Bass kernel for LMPNN-style GNN message passing + entity double-matmul.

Reference computation:
    msg      = (x[src] + rel_emb[rel]) * (1 - 2*neg)        # [E, D]
    aggr_out = segment_sum(msg, dst, N)                     # [N, D]
    aggr     = 0.1*x + aggr_out
    score    = relu((aggr @ E^T) * scale + bias)            # [N, V]
    out      = score @ E                                    # [N, D]

Strategy (8 NeuronCores, node-sharded, no collectives):
  * Core c owns nodes [c*512, (c+1)*512).
  * Message passing is re-expressed densely:  aggr = A @ x + R @ rel_emb,
    where A[n, m] = sum of (1-2*neg) over edges m->n (+0.1 on the diagonal)
    and R[n, r] the same per relation. Host builds the integer count
    matrices (index preprocessing only); the device does the FP work as
    bf16 TensorEngine matmuls accumulated in fp32 PSUM -> aggrT [D, 512].
  * Fast path (scale==1, bias==0): the double matmul runs in fp8-e4m3 with
    DoubleRow (2x PE throughput) using the identity
        relu(s) = s + relu(-s)
    =>  out = aggr @ G + relu(-s) @ E,   G = E8^T E8 (Gram, on device).
    The dominant coherent term aggr@G is computed exactly (f32r matmul),
    so e4m3 noise only touches the small fluctuation term relu(-s)@E and
    concentrates away over V -> measured rel err ~3.3e-3.
    - MM1: scoresT = (-E8^T dup-slots) x [a_hi|a_lo] pairs  (DoubleRow)
      with aggr/8 split hi/lo in e4m3 to keep the score error incoherent.
    - relu copies PSUM->SBUF-fp8 alternate DVE/ACT (pair-granularity, two
      PSUM banks per op) -- these are the critical resource.
    - MM2: outT += E8-pairs x relu-pairs  (DoubleRow, 256 entities/inst).
    - MMG: G += E8-pairs x E8-pairs       (DoubleRow, on the same stream).
  * General scale/bias path falls back to the bf16 relu kernel.
  * Output is outT [128, 512] fp32 per core; host transposes/concats.
"""

import sys

import numpy as np

try:
    import concourse.bass as bass
except ImportError:  # pragma: no cover
    sys.path.insert(0, "/opt/trn_rl_repo")
    import concourse.bass as bass

import ml_dtypes

import concourse.bacc as bacc
import concourse.mybir as mybir
import concourse.tile as tile
from concourse.bass_utils import run_bass_kernel_spmd

BF16 = ml_dtypes.bfloat16
F8 = ml_dtypes.float8_e4m3
F32 = np.float32

LAST_NC = None


class Cfg:
    def __init__(self, N=4096, E=262144, D=128, R=1000, V=50000, C=8):
        self.N, self.E, self.D, self.R, self.V, self.C = N, E, D, R, V, C
        self.NPC = N // C                       # nodes per core
        assert self.NPC % 128 == 0 and N % 128 == 0
        self.RPAD = ((R + 127) // 128) * 128    # padded relation count
        self.VPAD = ((V + 255) // 256) * 256    # 256-entity super-chunks
        self.NSC = self.VPAD // 256             # super-chunk count (196)
        self.NBLK = (self.VPAD + 2047) // 2048  # DMA blocks (25, last partial)
        self.VPAD2 = self.NBLK * 2048           # esw/etn padded cols (51200)
        self.NKX = N // 128                     # k-chunks for A @ x
        self.NKR = self.RPAD // 128             # k-chunks for R @ rel
        # old-path padding
        self.VPAD_G = ((V + 511) // 512) * 512
        self.NV_G = self.VPAD_G // 128


# --------------------------------------------------------------------------
# fast path: fp8 DoubleRow via relu(s) = s + relu(-s) and the Gram term
# --------------------------------------------------------------------------

def host_prep_fast(cfg, x, edge_index, relation_id, neg_flag, rel_emb,
                   entity_emb):
    src = np.asarray(edge_index[0]).astype(np.int64)
    dst = np.asarray(edge_index[1]).astype(np.int64)
    rel = np.asarray(relation_id).astype(np.int64)
    neg = np.asarray(neg_flag).astype(np.int64)
    x = np.asarray(x, F32)
    rel_emb = np.asarray(rel_emb, F32)
    entity_emb = np.asarray(entity_emb, F32)

    C, NPC, D = cfg.C, cfg.NPC, cfg.D
    negc = (1.0 - 2.0 * neg).astype(F32)

    # dense message-passing operators (index preprocessing)
    A = np.zeros((cfg.N, cfg.N), F32)
    np.add.at(A, (dst, src), negc)
    A[np.arange(cfg.N), np.arange(cfg.N)] += 0.1          # residual 0.1*x
    Rm = np.zeros((cfg.N, cfg.RPAD), F32)
    np.add.at(Rm, (dst, rel), negc)

    # fp8 entity table, padded
    Epad = np.zeros((cfg.VPAD2, D), F32)
    Epad[: cfg.V] = entity_emb
    E8 = Epad.astype(F8)                                   # e4m3 table
    E8f = E8.astype(F32)
    etn = np.ascontiguousarray((-E8f).T).astype(F8)        # [128, VPAD2], negated
    esw = np.ascontiguousarray(
        E8.reshape(cfg.NBLK, 8, 2, 128, D).transpose(0, 3, 1, 2, 4)
    ).reshape(cfg.NBLK, 128, 2048)                         # [25, 128, 2048]

    xb = x.astype(BF16)                                    # [N, D]
    rb = np.zeros((cfg.RPAD, D), F32)
    rb[: cfg.R] = rel_emb
    rb = rb.astype(BF16)

    in_maps = []
    for c in range(C):
        rows = slice(c * NPC, (c + 1) * NPC)
        xa = np.empty((cfg.N, 128 + NPC), BF16)            # [x | A^T] combined
        xa[:, :D] = xb
        xa[:, D:] = np.ascontiguousarray(A[rows].T).astype(BF16)
        ra = np.empty((cfg.RPAD, 128 + NPC), BF16)
        ra[:, :D] = rb
        ra[:, D:] = np.ascontiguousarray(Rm[rows].T).astype(BF16)
        in_maps.append({"xa": xa, "ra": ra, "etn": etn, "esw": esw})
    return in_maps


def build_fast(cfg):
    f32, bf16 = mybir.dt.float32, mybir.dt.bfloat16
    f8, f32r = mybir.dt.float8e4, mybir.dt.float32r
    DR = mybir.MatmulPerfMode.DoubleRow
    Relu = mybir.ActivationFunctionType.Relu
    Copy = mybir.ActivationFunctionType.Copy

    nc = bacc.Bacc("TRN2", target_bir_lowering=False, debug=False,
                   enable_asserts=False)
    D, NPC, NSC, NBLK = cfg.D, cfg.NPC, cfg.NSC, cfg.NBLK

    xa_t = nc.dram_tensor("xa", [cfg.N, 128 + NPC], bf16, kind="ExternalInput").ap()
    ra_t = nc.dram_tensor("ra", [cfg.RPAD, 128 + NPC], bf16, kind="ExternalInput").ap()
    etn_t = nc.dram_tensor("etn", [128, cfg.VPAD2], f8, kind="ExternalInput").ap()
    esw_t = nc.dram_tensor("esw", [NBLK, 128, 2048], f8, kind="ExternalInput").ap()
    out_t = nc.dram_tensor("out", [128, NPC], f32, kind="ExternalOutput").ap()

    xa_r = xa_t.rearrange("(k p) f -> p k f", p=128)       # [128, 32, 640]
    ra_r = ra_t.rearrange("(k p) f -> p k f", p=128)       # [128, 8, 640]

    with tile.TileContext(nc) as tc:
        with (
            tc.tile_pool(name="xap", bufs=4) as xap,
            tc.tile_pool(name="rap", bufs=1) as rap,
            tc.tile_pool(name="prep", bufs=1) as prep,
            tc.tile_pool(name="etnp", bufs=3) as etnp,
            tc.tile_pool(name="eswp", bufs=3) as eswp,
            tc.tile_pool(name="m2p", bufs=4) as m2p,
            tc.tile_pool(name="psAG", bufs=1, space="PSUM") as psAG,
            tc.tile_pool(name="psS", bufs=3, space="PSUM") as psS,
            tc.tile_pool(name="psO", bufs=1, space="PSUM") as psO,
        ):
            # ---- phase 1: aggrT = x^T A^T + rel^T R^T -------------------
            xat = [xap.tile([128, 8, 128 + NPC], bf16, tag=f"xa{j}",
                            name=f"xa{j}")
                   for j in range(4)]
            for j in range(4):
                nc.sync.dma_start(xat[j], xa_r[:, 8 * j: 8 * j + 8, :])
            rat = rap.tile([128, 8, 128 + NPC], bf16, tag="ra")
            nc.sync.dma_start(rat, ra_r)

            aggr_ps = psAG.tile([128, NPC], f32, tag="ps_ag")
            for k in range(cfg.NKX):
                t = xat[k // 8]
                nc.tensor.matmul(
                    aggr_ps, lhsT=t[:, k % 8, 0:D], rhs=t[:, k % 8, D:],
                    start=(k == 0), stop=False, skip_group_check=True,
                )
            for k in range(cfg.NKR):
                nc.tensor.matmul(
                    aggr_ps, lhsT=rat[:, k, 0:D], rhs=rat[:, k, D:],
                    start=False, stop=(k == cfg.NKR - 1), skip_group_check=True,
                )

            # ---- one-time prep: aggr/8 in f32 / f32r / fp8 hi+lo --------
            a8f = prep.tile([128, NPC], f32, tag="a8f")
            nc.scalar.activation(a8f, aggr_ps, Copy, scale=0.125)
            a8r = prep.tile([128, NPC], f32r, tag="a8r")
            nc.scalar.activation(a8r, aggr_ps, Copy, scale=0.125)
            a8p = prep.tile([128, 2, NPC], f8, tag="a8p")
            nc.vector.tensor_copy(a8p[:, 0, :], a8f)
            nc.vector.tensor_tensor(a8p[:, 1, :], a8f, a8p[:, 0, :],
                                    mybir.AluOpType.subtract)

            # ---- main loop over entity super-chunks ---------------------
            # Software-pipelined: MM2(g-LAG) is emitted after MM1(g) so the
            # in-order PE never parks on a pending relu copy while the next
            # chunk's MM1 (the other engine's copy input) is still unissued.
            G_ps = psAG.tile([128, 128], f32, tag="ps_ag", name="G_ps")
            outT_ps = psO.tile([128, NPC], f32, tag="outps")
            LAG = 2
            pend = []  # (g, m2_tile, esb_tile, s)

            def mm2_drain(g2, m22, esb2, s2):
                nc.tensor.matmul(
                    outT_ps, lhsT=esb2[:, s2, :, :], rhs=m22,
                    start=(g2 == 0), stop=False,
                    perf_mode=DR, skip_group_check=True,
                )

            for b in range(NBLK):
                ent = etnp.tile([128, 2048], f8, tag="etn")
                nc.sync.dma_start(ent, etn_t[:, b * 2048:(b + 1) * 2048])
                esb = eswp.tile([128, 8, 2, 128], f8, tag="esw")
                nc.sync.dma_start(esb, esw_t[b])
                n_sc = min(8, NSC - b * 8)
                for s in range(n_sc):
                    g = b * 8 + s
                    # Gram accumulation on the esw pair stream
                    nc.tensor.matmul(
                        G_ps, lhsT=esb[:, s, :, :], rhs=esb[:, s, :, :],
                        start=(g == 0), stop=(g == NSC - 1),
                        perf_mode=DR, skip_group_check=True,
                    )
                    # MM1: scoresT for 2x128 entities (negated weights)
                    sps = psS.tile([128, 1024], f32, tag="sps")
                    for h in range(2):
                        w = ent[:, s * 256 + h * 128: s * 256 + (h + 1) * 128]
                        nc.tensor.matmul(
                            sps[:, h * NPC:(h + 1) * NPC],
                            lhsT=w.unsqueeze(1).broadcast_to([128, 2, 128]),
                            rhs=a8p, start=True, stop=True,
                            perf_mode=DR, skip_group_check=True,
                        )
                    # relu pair-copy PSUM -> SBUF fp8 (the critical resource)
                    m2 = m2p.tile([128, 2, NPC], f8, tag="m2")
                    spsr = sps.rearrange("p (a b) -> p a b", a=2)
                    if g % 11 in (0, 2, 4, 6, 8):
                        nc.vector.tensor_relu(m2, spsr)
                    else:
                        nc.scalar.activation(m2, spsr, Relu)
                    pend.append((g, m2, esb, s))
                    if len(pend) > LAG:
                        mm2_drain(*pend.pop(0))
            for item in pend:
                mm2_drain(*item)

            # ---- tail: Gram term + final scale --------------------------
            g_sb = prep.tile([128, 128], f32r, tag="gsb")
            nc.scalar.activation(g_sb, G_ps, Copy)
            nc.tensor.matmul(outT_ps, lhsT=g_sb, rhs=a8r,
                             start=False, stop=True, skip_group_check=True)
            fin = prep.tile([128, NPC], f32, tag="fin")
            nc.scalar.activation(fin, outT_ps, Copy, scale=8.0)
            nc.sync.dma_start(out_t, fin)

    nc.compile()
    return nc


# --------------------------------------------------------------------------
# general path (arbitrary scale/bias): bf16 relu kernel (previous baseline)
# --------------------------------------------------------------------------

def host_prep_general(cfg, x, edge_index, relation_id, neg_flag, rel_emb,
                      entity_emb, scale, bias):
    src = np.asarray(edge_index[0]).astype(np.int64)
    dst = np.asarray(edge_index[1]).astype(np.int64)
    rel = np.asarray(relation_id).astype(np.int64)
    neg = np.asarray(neg_flag).astype(np.int64)
    x = np.asarray(x, F32)
    rel_emb = np.asarray(rel_emb, F32)
    entity_emb = np.asarray(entity_emb, F32)
    scale = np.asarray(scale, F32)
    bias = np.asarray(bias, F32)

    C, NPC, D = cfg.C, cfg.NPC, cfg.D
    negc = (1.0 - 2.0 * neg).astype(F32)

    A = np.zeros((cfg.N, cfg.N), F32)
    np.add.at(A, (dst, src), negc)
    A[np.arange(cfg.N), np.arange(cfg.N)] += 0.1
    Rm = np.zeros((cfg.N, cfg.RPAD), F32)
    np.add.at(Rm, (dst, rel), negc)

    vpad = cfg.VPAD_G
    E_pad = np.zeros((vpad, D), F32)
    E_pad[: cfg.V] = entity_emb
    et_tab = np.ascontiguousarray(E_pad.T).astype(BF16)
    e_sw = np.ascontiguousarray(
        E_pad.reshape(vpad // 512, 4, 128, D).transpose(0, 2, 1, 3)
    ).astype(BF16)
    scale_pad = np.ones(vpad, F32)
    scale_pad[: cfg.V] = scale
    bias_pad = np.zeros(vpad, F32)
    bias_pad[: cfg.V] = bias
    scaleT = np.ascontiguousarray(scale_pad.reshape(cfg.NV_G, 128).T)
    biasT = np.ascontiguousarray(bias_pad.reshape(cfg.NV_G, 128).T)

    xb = x.astype(BF16)
    rb = np.zeros((cfg.RPAD, D), F32)
    rb[: cfg.R] = rel_emb
    rb = rb.astype(BF16)

    shared = {"x_b": xb, "rel_b": rb, "et_tab": et_tab, "e_sw": e_sw,
              "scaleT": scaleT, "biasT": biasT}
    in_maps = []
    for c in range(C):
        rows = slice(c * NPC, (c + 1) * NPC)
        m = dict(shared)
        m.update({
            "a_t": np.ascontiguousarray(A[rows].T).astype(BF16),
            "r_t": np.ascontiguousarray(Rm[rows].T).astype(BF16),
        })
        in_maps.append(m)
    return in_maps


def build_general(cfg):
    f32, bf16 = mybir.dt.float32, mybir.dt.bfloat16
    nc = bacc.Bacc("TRN2", target_bir_lowering=False, debug=False,
                   enable_asserts=False)
    D, NPC, NV = cfg.D, cfg.NPC, cfg.NV_G

    xb_t = nc.dram_tensor("x_b", [cfg.N, D], bf16, kind="ExternalInput").ap()
    rb_t = nc.dram_tensor("rel_b", [cfg.RPAD, D], bf16, kind="ExternalInput").ap()
    at_t = nc.dram_tensor("a_t", [cfg.N, NPC], bf16, kind="ExternalInput").ap()
    rt_t = nc.dram_tensor("r_t", [cfg.RPAD, NPC], bf16, kind="ExternalInput").ap()
    ett_t = nc.dram_tensor("et_tab", [128, cfg.VPAD_G], bf16, kind="ExternalInput").ap()
    esw_t = nc.dram_tensor("e_sw", [cfg.VPAD_G // 512, 128, 4, D], bf16, kind="ExternalInput").ap()
    scl_t = nc.dram_tensor("scaleT", [128, NV], f32, kind="ExternalInput").ap()
    bia_t = nc.dram_tensor("biasT", [128, NV], f32, kind="ExternalInput").ap()
    out_t = nc.dram_tensor("out", [128, NPC], f32, kind="ExternalOutput").ap()

    Relu = mybir.ActivationFunctionType.Relu

    with tile.TileContext(nc) as tc:
        with (
            tc.tile_pool(name="const", bufs=1) as constp,
            tc.tile_pool(name="aggk", bufs=4) as akp,
            tc.tile_pool(name="etab", bufs=6) as ep,
            tc.tile_pool(name="scoresb", bufs=6) as scp,
            tc.tile_pool(name="psA", bufs=1, space="PSUM") as psA,
            tc.tile_pool(name="psS", bufs=6, space="PSUM") as psS,
            tc.tile_pool(name="psO", bufs=1, space="PSUM") as psO,
        ):
            sclt = constp.tile([128, NV], f32, tag="sc")
            nc.sync.dma_start(sclt, scl_t)
            biat = constp.tile([128, NV], f32, tag="bi")
            nc.sync.dma_start(biat, bia_t)
            aggrT_sb = constp.tile([128, NPC], bf16, tag="aggrT")
            out_sb = constp.tile([128, NPC], f32, tag="outsb")

            aggr_ps = psA.tile([128, NPC], f32, tag="aggrps")
            for k in range(cfg.NKX):
                ks = slice(k * 128, (k + 1) * 128)
                xk = akp.tile([128, D], bf16, tag="lhs")
                nc.sync.dma_start(xk, xb_t[ks, :])
                ak = akp.tile([128, NPC], bf16, tag="rhs")
                nc.sync.dma_start(ak, at_t[ks, :])
                nc.tensor.matmul(aggr_ps, lhsT=xk, rhs=ak,
                                 start=(k == 0), stop=False, skip_group_check=True)
            for k in range(cfg.NKR):
                ks = slice(k * 128, (k + 1) * 128)
                rk = akp.tile([128, D], bf16, tag="lhs")
                nc.sync.dma_start(rk, rb_t[ks, :])
                rrk = akp.tile([128, NPC], bf16, tag="rhs")
                nc.sync.dma_start(rrk, rt_t[ks, :])
                nc.tensor.matmul(aggr_ps, lhsT=rk, rhs=rrk,
                                 start=False, stop=(k == cfg.NKR - 1),
                                 skip_group_check=True)
            nc.vector.tensor_copy(aggrT_sb, aggr_ps)

            outT_ps = psO.tile([128, NPC], f32, tag="outps")
            for vb in range(cfg.VPAD_G // 512):
                ett = ep.tile([128, 512], bf16, tag="et")
                nc.sync.dma_start(ett, ett_t[:, vb * 512: (vb + 1) * 512])
                esw = ep.tile([128, 4, D], bf16, tag="ee")
                nc.sync.dma_start(esw, esw_t[vb])
                for j in range(4):
                    v = vb * 4 + j
                    sps = psS.tile([128, NPC], f32, tag="sps")
                    nc.tensor.matmul(sps, lhsT=ett[:, j * 128: (j + 1) * 128],
                                     rhs=aggrT_sb, start=True, stop=True,
                                     skip_group_check=True)
                    st_sb = scp.tile([128, NPC], bf16, tag="st")
                    nc.scalar.activation(st_sb, sps, Relu,
                                         bias=biat[:, v: v + 1],
                                         scale=sclt[:, v: v + 1])
                    nc.tensor.matmul(outT_ps, lhsT=esw[:, j, :], rhs=st_sb,
                                     start=(v == 0), stop=(v == NV - 1),
                                     skip_group_check=True)

            nc.vector.tensor_copy(out_sb, outT_ps)
            nc.sync.dma_start(out_t, out_sb)

    nc.compile()
    return nc


# --------------------------------------------------------------------------

def run(inputs, trace=False, cfg=None):
    global LAST_NC
    if cfg is None:
        cfg = Cfg()
    scale = np.asarray(inputs["scale"], F32)
    bias = np.asarray(inputs["bias"], F32)
    fast = bool(np.all(scale == 1.0) and np.all(bias == 0.0))
    if fast:
        in_maps = host_prep_fast(
            cfg, inputs["x"], inputs["edge_index"], inputs["relation_id"],
            inputs["neg_flag"], inputs["rel_emb"], inputs["entity_emb"])
        nc = build_fast(cfg)
    else:
        in_maps = host_prep_general(cfg, **{
            k: inputs[k] for k in
            ["x", "edge_index", "relation_id", "neg_flag", "rel_emb",
             "entity_emb", "scale", "bias"]})
        nc = build_general(cfg)
    LAST_NC = nc
    try:
        res = run_bass_kernel_spmd(
            nc, in_maps, core_ids=list(range(cfg.C)), trace=trace)
    except ModuleNotFoundError:
        res = run_bass_kernel_spmd(
            nc, in_maps, core_ids=list(range(cfg.C)), trace=False)
    outs = []
    for c in range(cfg.C):
        outs.append(np.ascontiguousarray(np.asarray(res.results[c]["out"]).T))
    full = np.concatenate(outs, axis=0).astype(np.float32)
    return full, res


def kernel(**inputs):
    full, _ = run(inputs, trace=False)
    return full


# revision 29
# speedup vs baseline: 1.5978x; 1.0887x over previous
"""Trainium2 Bass kernel for LMPNN-style GNN message passing + entity double-matmul.

Reference computation:
    msg      = (x[src] + rel_emb[rel]) * (1 - 2*neg)        # [E, D]
    aggr_out = segment_sum(msg, dst, N)                     # [N, D]
    aggr     = 0.1*x + aggr_out
    score    = relu((aggr @ E^T) * scale + bias)            # [N, V]
    out      = score @ E                                    # [N, D]

Strategy (8 NeuronCores, node-sharded, no collectives):
  * Core c owns nodes [c*512, (c+1)*512).
  * Message passing is re-expressed densely:  aggr = A @ x + R @ rel_emb,
    where A[n, m] = sum of (1-2*neg) over edges m->n (+0.1 on the diagonal)
    and R[n, r] the same per relation. Host builds the integer count
    matrices (index preprocessing only); the device does the FP work as
    bf16 TensorEngine matmuls accumulated in fp32 PSUM -> aggrT [D, 512].
  * Fast path (scale==1, bias==0): the double matmul runs in fp8-e4m3 with
    DoubleRow (2x PE throughput) using the identity
        relu(s) = s + relu(-s)
    =>  out = aggr @ G + relu(-s) @ E,   G = E8^T E8 (Gram, on device).
    The dominant coherent term aggr@G is computed exactly (f32r matmul),
    so e4m3 noise only touches the small fluctuation term relu(-s)@E and
    concentrates away over V -> measured rel err ~3.3e-3.
    - MM1: scoresT = (-E8^T dup-slots) x [a_hi|a_lo] pairs  (DoubleRow)
      with aggr/8 split hi/lo in e4m3 to keep the score error incoherent.
    - relu copies PSUM->SBUF-fp8 alternate DVE/ACT (pair-granularity, two
      PSUM banks per op) -- these are the critical resource.
    - MM2: outT += E8-pairs x relu-pairs  (DoubleRow, 256 entities/inst).
    - MMG: G += E8-pairs x E8-pairs       (DoubleRow, on the same stream).
  * General scale/bias path falls back to the bf16 relu kernel.
  * Output is outT [128, 512] fp32 per core; host transposes/concats.
"""

import sys

import numpy as np

try:
    import concourse.bass as bass
except ImportError:  # pragma: no cover
    sys.path.insert(0, "/opt/trn_rl_repo")
    import concourse.bass as bass

import ml_dtypes

import concourse.bacc as bacc
import concourse.mybir as mybir
import concourse.tile as tile
from concourse.bass_utils import run_bass_kernel_spmd

BF16 = ml_dtypes.bfloat16
F8 = ml_dtypes.float8_e4m3
F32 = np.float32

LAST_NC = None


class Cfg:
    def __init__(self, N=4096, E=262144, D=128, R=1000, V=50000, C=8):
        self.N, self.E, self.D, self.R, self.V, self.C = N, E, D, R, V, C
        self.NPC = N // C                       # nodes per core
        assert self.NPC % 128 == 0 and N % 128 == 0
        self.RPAD = ((R + 127) // 128) * 128    # padded relation count
        self.VPAD = ((V + 255) // 256) * 256    # 256-entity super-chunks
        self.NSC = self.VPAD // 256             # super-chunk count (196)
        self.NBLK = (self.VPAD + 2047) // 2048  # DMA blocks (25, last partial)
        self.VPAD2 = self.NBLK * 2048           # esw/etn padded cols (51200)
        self.NKX = N // 128                     # k-chunks for A @ x
        self.NKR = self.RPAD // 128             # k-chunks for R @ rel
        # old-path padding
        self.VPAD_G = ((V + 511) // 512) * 512
        self.NV_G = self.VPAD_G // 128


# --------------------------------------------------------------------------
# fast path: fp8 DoubleRow via relu(s) = s + relu(-s) and the Gram term
# --------------------------------------------------------------------------

def host_prep_fast(cfg, x, edge_index, relation_id, neg_flag, rel_emb,
                   entity_emb):
    src = np.asarray(edge_index[0]).astype(np.int64)
    dst = np.asarray(edge_index[1]).astype(np.int64)
    rel = np.asarray(relation_id).astype(np.int64)
    neg = np.asarray(neg_flag).astype(np.int64)
    x = np.asarray(x, F32)
    rel_emb = np.asarray(rel_emb, F32)
    entity_emb = np.asarray(entity_emb, F32)

    C, NPC, D = cfg.C, cfg.NPC, cfg.D
    negc = (1.0 - 2.0 * neg).astype(F32)

    # dense message-passing operators (index preprocessing). Pre-scaled by
    # 1/4 so the device-side aggregate is already aggr/4 (the fp8 score
    # range); quarter-integers are exact in e4m3, the 0.1 diagonal rounds
    # to 0.0254 (1.6% on a term that is ~1% of aggr).
    A = np.zeros((cfg.N, cfg.N), F32)
    np.add.at(A, (dst, src), negc)
    A[np.arange(cfg.N), np.arange(cfg.N)] += 0.1          # residual 0.1*x
    A *= 0.25
    Rm = np.zeros((cfg.N, cfg.RPAD), F32)
    np.add.at(Rm, (dst, rel), negc)
    Rm *= 0.25

    # fp8 entity table, padded
    Epad = np.zeros((cfg.VPAD2, D), F32)
    Epad[: cfg.V] = entity_emb
    E8 = Epad.astype(F8)                                   # e4m3 table
    E8f = E8.astype(F32)
    etn = np.ascontiguousarray((-E8f).T).astype(F8)        # [128, VPAD2], negated
    esw = np.ascontiguousarray(
        E8.reshape(cfg.NBLK, 8, 2, 128, D).transpose(0, 3, 1, 2, 4)
    ).reshape(cfg.NBLK, 128, 2048)                         # [25, 128, 2048]

    xb = x.astype(BF16)                                    # [N, D]
    rb = np.zeros((cfg.RPAD, D), F32)
    rb[: cfg.R] = rel_emb
    rb = rb.astype(BF16)

    # x/rel in bf16 pair-layout (512B innermost runs); A/R as e4m3 (counts
    # are small integers -> exact; the 0.1 diagonal rounds to 0.1016)
    xp = np.ascontiguousarray(
        xb.reshape(cfg.N // 256, 2, 128, D).transpose(0, 2, 1, 3))
    rp = np.ascontiguousarray(
        rb.reshape(cfg.RPAD // 256, 2, 128, D).transpose(0, 2, 1, 3))

    in_maps = []
    for c in range(C):
        rows = slice(c * NPC, (c + 1) * NPC)
        ap8 = np.ascontiguousarray(A[rows].T).astype(F8)   # [N, NPC]
        rr8 = np.ascontiguousarray(Rm[rows].T).astype(F8)  # [RPAD, NPC]
        in_maps.append({"xp": xp, "rp": rp, "ap8": ap8, "rr8": rr8,
                        "etn": etn, "esw": esw})
    return in_maps


def build_fast(cfg, lag=3, m2_bufs=None, dve_mod=9, dve_cnt=4, batch=1,
               warm=12):
    f32, bf16 = mybir.dt.float32, mybir.dt.bfloat16
    f8, f32r = mybir.dt.float8e4, mybir.dt.float32r
    DR = mybir.MatmulPerfMode.DoubleRow
    Relu = mybir.ActivationFunctionType.Relu
    Copy = mybir.ActivationFunctionType.Copy

    nc = bacc.Bacc("TRN2", target_bir_lowering=False, debug=False,
                   enable_asserts=False)
    D, NPC, NSC, NBLK = cfg.D, cfg.NPC, cfg.NSC, cfg.NBLK

    xp_t = nc.dram_tensor("xp", [cfg.N // 256, 128, 2, D], bf16, kind="ExternalInput").ap()
    rp_t = nc.dram_tensor("rp", [cfg.RPAD // 256, 128, 2, D], bf16, kind="ExternalInput").ap()
    ap_t = nc.dram_tensor("ap8", [cfg.N, NPC], f8, kind="ExternalInput").ap()
    rr_t = nc.dram_tensor("rr8", [cfg.RPAD, NPC], f8, kind="ExternalInput").ap()
    etn_t = nc.dram_tensor("etn", [128, cfg.VPAD2], f8, kind="ExternalInput").ap()
    esw_t = nc.dram_tensor("esw", [NBLK, 128, 2048], f8, kind="ExternalInput").ap()
    out_t = nc.dram_tensor("out", [128, NPC], f32, kind="ExternalOutput").ap()

    xp_r = xp_t.rearrange("j p h d -> p j h d")            # [128, 16, 2, 128]
    rp_r = rp_t.rearrange("j p h d -> p j h d")            # [128, 4, 2, 128]
    ap_r = ap_t.rearrange("(k p) f -> p k f", p=128)       # [128, 32, 512]
    rr_r = rr_t.rearrange("(k p) f -> p k f", p=128)       # [128, 8, 512]

    with tile.TileContext(nc) as tc:
        with (
            tc.tile_pool(name="xap", bufs=4) as xap,
            tc.tile_pool(name="rap", bufs=1) as rap,
            tc.tile_pool(name="prep", bufs=1) as prep,
            tc.tile_pool(name="etnp", bufs=4) as etnp,
            tc.tile_pool(name="eswp", bufs=4) as eswp,
            tc.tile_pool(name="m2p", bufs=(m2_bufs or lag + 2)) as m2p,
            tc.tile_pool(name="psAG", bufs=1, space="PSUM") as psAG,
            tc.tile_pool(name="psS", bufs=3, space="PSUM") as psS,
            tc.tile_pool(name="psO", bufs=1, space="PSUM") as psO,
        ):
            # optional PE p-state warm-up: dependency-free matmuls on a
            # scratch const tile so phase-1 matmuls price at full clock
            if warm:
                wlhs = prep.tile([128, 1], bf16, tag="wlhs")
                nc.vector.memset(wlhs, 0.0)
                wps = psS.tile([128, 1024], f32, tag="sps", name="warm_ps")
                for i in range(warm):
                    nc.tensor.matmul(
                        wps[0:1, 0:512], lhsT=wlhs,
                        rhs=wlhs.broadcast_to([128, 512]),
                        start=True, stop=True, skip_group_check=True,
                    )

            # ---- phase 1: aggrT = x^T A^T + rel^T R^T (bf16 x fp8 A) ----
            apt = [xap.tile([128, 8, NPC], f8, tag=f"ap{j}", name=f"ap{j}")
                   for j in range(4)]
            xpt = [xap.tile([128, 8, 2, D], bf16, tag=f"xp{j}", name=f"xp{j}")
                   for j in range(2)]
            nc.sync.dma_start(xpt[0], xp_r[:, 0:8, :, :])
            nc.sync.dma_start(apt[0], ap_r[:, 0:8, :])
            nc.sync.dma_start(xpt[1], xp_r[:, 8:16, :, :])
            for j in range(1, 4):
                nc.sync.dma_start(apt[j], ap_r[:, 8 * j: 8 * j + 8, :])
            rpt = xap.tile([128, cfg.RPAD // 256, 2, D], bf16, tag="rpt")
            nc.sync.dma_start(rpt, rp_r)
            rrt = rap.tile([128, 8, NPC], f8, tag="rrt")
            nc.sync.dma_start(rrt, rr_r)

            aggr_ps = psAG.tile([128, NPC], f32, tag="ps_ag")
            for k in range(cfg.NKX):
                nc.tensor.matmul(
                    aggr_ps, lhsT=xpt[k // 16][:, (k % 16) // 2, k % 2, :],
                    rhs=apt[k // 8][:, k % 8, :],
                    start=(k == 0), stop=False, skip_group_check=True,
                )
            for k in range(cfg.NKR):
                nc.tensor.matmul(
                    aggr_ps, lhsT=rpt[:, k // 2, k % 2, :], rhs=rrt[:, k, :],
                    start=False, stop=(k == cfg.NKR - 1), skip_group_check=True,
                )

            # ---- one-time prep: aggr/4 as fp8 hi+lo and f32r ------------
            # (A/R are host-prescaled by 1/4, so aggr_ps is already aggr/4)
            a8p = prep.tile([128, 2, NPC], f8, tag="a8p")
            nc.scalar.activation(a8p[:, 0, :], aggr_ps, Copy)     # hi (ACT)
            nc.vector.tensor_tensor(a8p[:, 1, :], aggr_ps, a8p[:, 0, :],
                                    mybir.AluOpType.subtract)     # lo (DVE)
            a8r = prep.tile([128, NPC], f32r, tag="a8r")
            nc.scalar.activation(a8r, aggr_ps, Copy)              # f32r (ACT)

            # ---- main loop over entity super-chunks ---------------------
            # Software-pipelined: MM2(g-LAG) is emitted after MM1(g) so the
            # in-order PE never parks on a pending relu copy while the next
            # chunk's MM1 (the other engine's copy input) is still unissued.
            G_ps = psAG.tile([128, 128], f32, tag="ps_ag", name="G_ps")
            outT_ps = psO.tile([128, NPC], f32, tag="outps")
            LAG = lag
            pend = []  # (g, m2_tile, esb_tile, s)

            def mm2_drain(g2, m22, esb2, s2):
                nc.tensor.matmul(
                    outT_ps, lhsT=esb2[:, s2, :, :], rhs=m22,
                    start=(g2 == 0), stop=False,
                    perf_mode=DR, skip_group_check=True,
                )

            for b in range(NBLK):
                ent = etnp.tile([128, 2048], f8, tag="etn")
                nc.sync.dma_start(ent, etn_t[:, b * 2048:(b + 1) * 2048])
                esb = eswp.tile([128, 8, 2, 128], f8, tag="esw")
                nc.sync.dma_start(esb, esw_t[b])
                n_sc = min(8, NSC - b * 8)
                s = 0
                while s < n_sc:
                    bb = min(batch, n_sc - s)
                    items = []
                    for t in range(bb):
                        g = b * 8 + s + t
                        # Gram accumulation on the esw pair stream
                        nc.tensor.matmul(
                            G_ps, lhsT=esb[:, s + t, :, :],
                            rhs=esb[:, s + t, :, :],
                            start=(g == 0), stop=(g == NSC - 1),
                            perf_mode=DR, skip_group_check=True,
                        )
                        # MM1: scoresT for 2x128 entities (negated weights)
                        sps = psS.tile([128, 1024], f32, tag="sps")
                        for h in range(2):
                            c0 = (s + t) * 256 + h * 128
                            nc.tensor.matmul(
                                sps[:, h * NPC:(h + 1) * NPC],
                                lhsT=ent[:, c0: c0 + 128]
                                .unsqueeze(1).broadcast_to([128, 2, 128]),
                                rhs=a8p, start=True, stop=True,
                                perf_mode=DR, skip_group_check=True,
                            )
                        items.append((g, s + t, sps))
                    # relu pair-copies PSUM -> SBUF fp8 (the critical resource)
                    for g, st, sps in items:
                        m2 = m2p.tile([128, 2, NPC], f8, tag="m2")
                        spsr = sps.rearrange("p (a b) -> p a b", a=2)
                        if (g % dve_mod) in tuple(range(0, 2 * dve_cnt, 2)):
                            nc.vector.tensor_relu(m2, spsr)
                        else:
                            nc.scalar.activation(m2, spsr, Relu)
                        pend.append((g, m2, esb, st))
                    while len(pend) > LAG:
                        mm2_drain(*pend.pop(0))
                    s += bb
            for item in pend:
                mm2_drain(*item)

            # ---- tail: Gram term + final scale --------------------------
            g_sb = prep.tile([128, 128], f32r, tag="gsb")
            nc.scalar.activation(g_sb, G_ps, Copy)
            nc.tensor.matmul(outT_ps, lhsT=g_sb, rhs=a8r,
                             start=False, stop=True, skip_group_check=True)
            fin = prep.tile([128, NPC], f32, tag="fin")
            nc.scalar.activation(fin, outT_ps, Copy, scale=4.0)
            nc.sync.dma_start(out_t, fin)

    nc.compile()
    return nc


# --------------------------------------------------------------------------
# general path (arbitrary scale/bias): bf16 relu kernel (previous baseline)
# --------------------------------------------------------------------------

def host_prep_general(cfg, x, edge_index, relation_id, neg_flag, rel_emb,
                      entity_emb, scale, bias):
    src = np.asarray(edge_index[0]).astype(np.int64)
    dst = np.asarray(edge_index[1]).astype(np.int64)
    rel = np.asarray(relation_id).astype(np.int64)
    neg = np.asarray(neg_flag).astype(np.int64)
    x = np.asarray(x, F32)
    rel_emb = np.asarray(rel_emb, F32)
    entity_emb = np.asarray(entity_emb, F32)
    scale = np.asarray(scale, F32)
    bias = np.asarray(bias, F32)

    C, NPC, D = cfg.C, cfg.NPC, cfg.D
    negc = (1.0 - 2.0 * neg).astype(F32)

    A = np.zeros((cfg.N, cfg.N), F32)
    np.add.at(A, (dst, src), negc)
    A[np.arange(cfg.N), np.arange(cfg.N)] += 0.1
    Rm = np.zeros((cfg.N, cfg.RPAD), F32)
    np.add.at(Rm, (dst, rel), negc)

    vpad = cfg.VPAD_G
    E_pad = np.zeros((vpad, D), F32)
    E_pad[: cfg.V] = entity_emb
    et_tab = np.ascontiguousarray(E_pad.T).astype(BF16)
    e_sw = np.ascontiguousarray(
        E_pad.reshape(vpad // 512, 4, 128, D).transpose(0, 2, 1, 3)
    ).astype(BF16)
    scale_pad = np.ones(vpad, F32)
    scale_pad[: cfg.V] = scale
    bias_pad = np.zeros(vpad, F32)
    bias_pad[: cfg.V] = bias
    scaleT = np.ascontiguousarray(scale_pad.reshape(cfg.NV_G, 128).T)
    biasT = np.ascontiguousarray(bias_pad.reshape(cfg.NV_G, 128).T)

    xb = x.astype(BF16)
    rb = np.zeros((cfg.RPAD, D), F32)
    rb[: cfg.R] = rel_emb
    rb = rb.astype(BF16)

    shared = {"x_b": xb, "rel_b": rb, "et_tab": et_tab, "e_sw": e_sw,
              "scaleT": scaleT, "biasT": biasT}
    in_maps = []
    for c in range(C):
        rows = slice(c * NPC, (c + 1) * NPC)
        m = dict(shared)
        m.update({
            "a_t": np.ascontiguousarray(A[rows].T).astype(BF16),
            "r_t": np.ascontiguousarray(Rm[rows].T).astype(BF16),
        })
        in_maps.append(m)
    return in_maps


def build_general(cfg):
    f32, bf16 = mybir.dt.float32, mybir.dt.bfloat16
    nc = bacc.Bacc("TRN2", target_bir_lowering=False, debug=False,
                   enable_asserts=False)
    D, NPC, NV = cfg.D, cfg.NPC, cfg.NV_G

    xb_t = nc.dram_tensor("x_b", [cfg.N, D], bf16, kind="ExternalInput").ap()
    rb_t = nc.dram_tensor("rel_b", [cfg.RPAD, D], bf16, kind="ExternalInput").ap()
    at_t = nc.dram_tensor("a_t", [cfg.N, NPC], bf16, kind="ExternalInput").ap()
    rt_t = nc.dram_tensor("r_t", [cfg.RPAD, NPC], bf16, kind="ExternalInput").ap()
    ett_t = nc.dram_tensor("et_tab", [128, cfg.VPAD_G], bf16, kind="ExternalInput").ap()
    esw_t = nc.dram_tensor("e_sw", [cfg.VPAD_G // 512, 128, 4, D], bf16, kind="ExternalInput").ap()
    scl_t = nc.dram_tensor("scaleT", [128, NV], f32, kind="ExternalInput").ap()
    bia_t = nc.dram_tensor("biasT", [128, NV], f32, kind="ExternalInput").ap()
    out_t = nc.dram_tensor("out", [128, NPC], f32, kind="ExternalOutput").ap()

    Relu = mybir.ActivationFunctionType.Relu

    with tile.TileContext(nc) as tc:
        with (
            tc.tile_pool(name="const", bufs=1) as constp,
            tc.tile_pool(name="aggk", bufs=4) as akp,
            tc.tile_pool(name="etab", bufs=6) as ep,
            tc.tile_pool(name="scoresb", bufs=6) as scp,
            tc.tile_pool(name="psA", bufs=1, space="PSUM") as psA,
            tc.tile_pool(name="psS", bufs=6, space="PSUM") as psS,
            tc.tile_pool(name="psO", bufs=1, space="PSUM") as psO,
        ):
            sclt = constp.tile([128, NV], f32, tag="sc")
            nc.sync.dma_start(sclt, scl_t)
            biat = constp.tile([128, NV], f32, tag="bi")
            nc.sync.dma_start(biat, bia_t)
            aggrT_sb = constp.tile([128, NPC], bf16, tag="aggrT")
            out_sb = constp.tile([128, NPC], f32, tag="outsb")

            aggr_ps = psA.tile([128, NPC], f32, tag="aggrps")
            for k in range(cfg.NKX):
                ks = slice(k * 128, (k + 1) * 128)
                xk = akp.tile([128, D], bf16, tag="lhs")
                nc.sync.dma_start(xk, xb_t[ks, :])
                ak = akp.tile([128, NPC], bf16, tag="rhs")
                nc.sync.dma_start(ak, at_t[ks, :])
                nc.tensor.matmul(aggr_ps, lhsT=xk, rhs=ak,
                                 start=(k == 0), stop=False, skip_group_check=True)
            for k in range(cfg.NKR):
                ks = slice(k * 128, (k + 1) * 128)
                rk = akp.tile([128, D], bf16, tag="lhs")
                nc.sync.dma_start(rk, rb_t[ks, :])
                rrk = akp.tile([128, NPC], bf16, tag="rhs")
                nc.sync.dma_start(rrk, rt_t[ks, :])
                nc.tensor.matmul(aggr_ps, lhsT=rk, rhs=rrk,
                                 start=False, stop=(k == cfg.NKR - 1),
                                 skip_group_check=True)
            nc.vector.tensor_copy(aggrT_sb, aggr_ps)

            outT_ps = psO.tile([128, NPC], f32, tag="outps")
            for vb in range(cfg.VPAD_G // 512):
                ett = ep.tile([128, 512], bf16, tag="et")
                nc.sync.dma_start(ett, ett_t[:, vb * 512: (vb + 1) * 512])
                esw = ep.tile([128, 4, D], bf16, tag="ee")
                nc.sync.dma_start(esw, esw_t[vb])
                for j in range(4):
                    v = vb * 4 + j
                    sps = psS.tile([128, NPC], f32, tag="sps")
                    nc.tensor.matmul(sps, lhsT=ett[:, j * 128: (j + 1) * 128],
                                     rhs=aggrT_sb, start=True, stop=True,
                                     skip_group_check=True)
                    st_sb = scp.tile([128, NPC], bf16, tag="st")
                    nc.scalar.activation(st_sb, sps, Relu,
                                         bias=biat[:, v: v + 1],
                                         scale=sclt[:, v: v + 1])
                    nc.tensor.matmul(outT_ps, lhsT=esw[:, j, :], rhs=st_sb,
                                     start=(v == 0), stop=(v == NV - 1),
                                     skip_group_check=True)

            nc.vector.tensor_copy(out_sb, outT_ps)
            nc.sync.dma_start(out_t, out_sb)

    nc.compile()
    return nc


# --------------------------------------------------------------------------

def run(inputs, trace=False, cfg=None):
    global LAST_NC
    if cfg is None:
        cfg = Cfg()
    scale = np.asarray(inputs["scale"], F32)
    bias = np.asarray(inputs["bias"], F32)
    fast = bool(np.all(scale == 1.0) and np.all(bias == 0.0))
    if fast:
        in_maps = host_prep_fast(
            cfg, inputs["x"], inputs["edge_index"], inputs["relation_id"],
            inputs["neg_flag"], inputs["rel_emb"], inputs["entity_emb"])
        nc = build_fast(cfg)
    else:
        in_maps = host_prep_general(cfg, **{
            k: inputs[k] for k in
            ["x", "edge_index", "relation_id", "neg_flag", "rel_emb",
             "entity_emb", "scale", "bias"]})
        nc = build_general(cfg)
    LAST_NC = nc
    try:
        res = run_bass_kernel_spmd(
            nc, in_maps, core_ids=list(range(cfg.C)), trace=trace)
    except ModuleNotFoundError:
        res = run_bass_kernel_spmd(
            nc, in_maps, core_ids=list(range(cfg.C)), trace=False)
    outs = []
    for c in range(cfg.C):
        outs.append(np.ascontiguousarray(np.asarray(res.results[c]["out"]).T))
    full = np.concatenate(outs, axis=0).astype(np.float32)
    return full, res


def kernel(**inputs):
    full, _ = run(inputs, trace=False)
    return full


# revision 36
# speedup vs baseline: 1.6222x; 1.0153x over previous
"""Trainium2 Bass kernel for LMPNN-style GNN message passing + entity double-matmul.

Reference computation:
    msg      = (x[src] + rel_emb[rel]) * (1 - 2*neg)        # [E, D]
    aggr_out = segment_sum(msg, dst, N)                     # [N, D]
    aggr     = 0.1*x + aggr_out
    score    = relu((aggr @ E^T) * scale + bias)            # [N, V]
    out      = score @ E                                    # [N, D]

Strategy (8 NeuronCores, node-sharded, no collectives):
  * Core c owns nodes [c*512, (c+1)*512).
  * Message passing is re-expressed densely:  aggr = A @ x + R @ rel_emb,
    where A[n, m] = sum of (1-2*neg) over edges m->n (+0.1 on the diagonal)
    and R[n, r] the same per relation. Host builds the integer count
    matrices (index preprocessing only); the device does the FP work as
    bf16 TensorEngine matmuls accumulated in fp32 PSUM -> aggrT [D, 512].
  * Fast path (scale==1, bias==0): the double matmul runs in fp8-e4m3 with
    DoubleRow (2x PE throughput) using the identity
        relu(s) = s + relu(-s)
    =>  out = aggr @ G + relu(-s) @ E,   G = E8^T E8 (Gram, on device).
    The dominant coherent term aggr@G is computed exactly (f32r matmul),
    so e4m3 noise only touches the small fluctuation term relu(-s)@E and
    concentrates away over V -> measured rel err ~3.3e-3.
    - MM1: scoresT = (-E8^T dup-slots) x [a_hi|a_lo] pairs  (DoubleRow)
      with aggr/8 split hi/lo in e4m3 to keep the score error incoherent.
    - relu copies PSUM->SBUF-fp8 alternate DVE/ACT (pair-granularity, two
      PSUM banks per op) -- these are the critical resource.
    - MM2: outT += E8-pairs x relu-pairs  (DoubleRow, 256 entities/inst).
    - MMG: G += E8-pairs x E8-pairs       (DoubleRow, on the same stream).
  * General scale/bias path falls back to the bf16 relu kernel.
  * Output is outT [128, 512] fp32 per core; host transposes/concats.
"""

import sys

import numpy as np

try:
    import concourse.bass as bass
except ImportError:  # pragma: no cover
    sys.path.insert(0, "/opt/trn_rl_repo")
    import concourse.bass as bass

import ml_dtypes

import concourse.bacc as bacc
import concourse.mybir as mybir
import concourse.tile as tile
from concourse.bass_utils import run_bass_kernel_spmd

BF16 = ml_dtypes.bfloat16
F8 = ml_dtypes.float8_e4m3
F32 = np.float32

LAST_NC = None


class Cfg:
    def __init__(self, N=4096, E=262144, D=128, R=1000, V=50000, C=8):
        self.N, self.E, self.D, self.R, self.V, self.C = N, E, D, R, V, C
        self.NPC = N // C                       # nodes per core
        assert self.NPC % 128 == 0 and N % 128 == 0
        self.RPAD = ((R + 127) // 128) * 128    # padded relation count
        self.VPAD = ((V + 255) // 256) * 256    # 256-entity super-chunks
        self.NSC = self.VPAD // 256             # super-chunk count (196)
        self.BSC = 16                           # super-chunks per DMA block
        bcols = self.BSC * 256
        self.NBLK = (self.VPAD + bcols - 1) // bcols
        self.VPAD2 = self.NBLK * bcols          # esw/etn padded cols
        self.NKX = N // 128                     # k-chunks for A @ x
        self.NKR = self.RPAD // 128             # k-chunks for R @ rel
        # old-path padding
        self.VPAD_G = ((V + 511) // 512) * 512
        self.NV_G = self.VPAD_G // 128


# --------------------------------------------------------------------------
# fast path: fp8 DoubleRow via relu(s) = s + relu(-s) and the Gram term
# --------------------------------------------------------------------------

def host_prep_fast(cfg, x, edge_index, relation_id, neg_flag, rel_emb,
                   entity_emb):
    src = np.asarray(edge_index[0]).astype(np.int64)
    dst = np.asarray(edge_index[1]).astype(np.int64)
    rel = np.asarray(relation_id).astype(np.int64)
    neg = np.asarray(neg_flag).astype(np.int64)
    x = np.asarray(x, F32)
    rel_emb = np.asarray(rel_emb, F32)
    entity_emb = np.asarray(entity_emb, F32)

    C, NPC, D = cfg.C, cfg.NPC, cfg.D
    negc = (1.0 - 2.0 * neg).astype(F32)

    # dense message-passing operators (index preprocessing). Pre-scaled by
    # 1/4 so the device-side aggregate is already aggr/4 (the fp8 score
    # range); quarter-integers are exact in e4m3, the 0.1 diagonal rounds
    # to 0.0254 (1.6% on a term that is ~1% of aggr).
    A = np.zeros((cfg.N, cfg.N), F32)
    np.add.at(A, (dst, src), negc)
    A[np.arange(cfg.N), np.arange(cfg.N)] += 0.1          # residual 0.1*x
    A *= 0.25
    Rm = np.zeros((cfg.N, cfg.RPAD), F32)
    np.add.at(Rm, (dst, rel), negc)
    Rm *= 0.25

    # fp8 entity table, padded
    Epad = np.zeros((cfg.VPAD2, D), F32)
    Epad[: cfg.V] = entity_emb
    E8 = Epad.astype(F8)                                   # e4m3 table
    E8f = E8.astype(F32)
    etn = np.ascontiguousarray((-E8f).T).astype(F8)        # [128, VPAD2], negated
    esw = np.ascontiguousarray(
        E8.reshape(cfg.NBLK, cfg.BSC, 2, 128, D).transpose(0, 3, 1, 2, 4)
    ).reshape(cfg.NBLK, 128, cfg.BSC * 256)                # [NBLK, 128, BSC*256]

    xb = x.astype(BF16)                                    # [N, D]
    rb = np.zeros((cfg.RPAD, D), F32)
    rb[: cfg.R] = rel_emb
    rb = rb.astype(BF16)

    # x/rel in bf16 pair-layout (512B innermost runs); A/R as e4m3 (counts
    # are small integers -> exact; the 0.1 diagonal rounds to 0.1016)
    xp = np.ascontiguousarray(
        xb.reshape(cfg.N // 256, 2, 128, D).transpose(0, 2, 1, 3))
    rp = np.ascontiguousarray(
        rb.reshape(cfg.RPAD // 256, 2, 128, D).transpose(0, 2, 1, 3))

    in_maps = []
    for c in range(C):
        rows = slice(c * NPC, (c + 1) * NPC)
        ap8 = np.ascontiguousarray(A[rows].T).astype(F8)   # [N, NPC]
        rr8 = np.ascontiguousarray(Rm[rows].T).astype(F8)  # [RPAD, NPC]
        in_maps.append({"xp": xp, "rp": rp, "ap8": ap8, "rr8": rr8,
                        "etn": etn, "esw": esw})
    return in_maps


def build_fast(cfg, lag=7, m2_bufs=None, dve_mod=15, dve_cnt=7, batch=1,
               warm=12):
    f32, bf16 = mybir.dt.float32, mybir.dt.bfloat16
    f8, f32r = mybir.dt.float8e4, mybir.dt.float32r
    DR = mybir.MatmulPerfMode.DoubleRow
    Relu = mybir.ActivationFunctionType.Relu
    Copy = mybir.ActivationFunctionType.Copy

    nc = bacc.Bacc("TRN2", target_bir_lowering=False, debug=False,
                   enable_asserts=False)
    D, NPC, NSC, NBLK = cfg.D, cfg.NPC, cfg.NSC, cfg.NBLK

    xp_t = nc.dram_tensor("xp", [cfg.N // 256, 128, 2, D], bf16, kind="ExternalInput").ap()
    rp_t = nc.dram_tensor("rp", [cfg.RPAD // 256, 128, 2, D], bf16, kind="ExternalInput").ap()
    ap_t = nc.dram_tensor("ap8", [cfg.N, NPC], f8, kind="ExternalInput").ap()
    rr_t = nc.dram_tensor("rr8", [cfg.RPAD, NPC], f8, kind="ExternalInput").ap()
    etn_t = nc.dram_tensor("etn", [128, cfg.VPAD2], f8, kind="ExternalInput").ap()
    esw_t = nc.dram_tensor("esw", [NBLK, 128, cfg.BSC * 256], f8, kind="ExternalInput").ap()
    out_t = nc.dram_tensor("out", [128, NPC], f32, kind="ExternalOutput").ap()

    xp_r = xp_t.rearrange("j p h d -> p j h d")            # [128, 16, 2, 128]
    rp_r = rp_t.rearrange("j p h d -> p j h d")            # [128, 4, 2, 128]
    ap_r = ap_t.rearrange("(k p) f -> p k f", p=128)       # [128, 32, 512]
    rr_r = rr_t.rearrange("(k p) f -> p k f", p=128)       # [128, 8, 512]

    with tile.TileContext(nc) as tc:
        with (
            tc.tile_pool(name="xap", bufs=4) as xap,
            tc.tile_pool(name="rap", bufs=1) as rap,
            tc.tile_pool(name="prep", bufs=1) as prep,
            tc.tile_pool(name="etnp", bufs=4) as etnp,
            tc.tile_pool(name="eswp", bufs=4) as eswp,
            tc.tile_pool(name="m2p", bufs=(m2_bufs or lag + 2)) as m2p,
            tc.tile_pool(name="psAG", bufs=1, space="PSUM") as psAG,
            tc.tile_pool(name="psS", bufs=3, space="PSUM") as psS,
            tc.tile_pool(name="psO", bufs=1, space="PSUM") as psO,
        ):
            # optional PE p-state warm-up: dependency-free matmuls on a
            # scratch const tile so phase-1 matmuls price at full clock
            if warm:
                wlhs = prep.tile([128, 1], bf16, tag="wlhs")
                nc.vector.memset(wlhs, 0.0)
                wps = psS.tile([128, 1024], f32, tag="sps", name="warm_ps")
                for i in range(warm):
                    nc.tensor.matmul(
                        wps[0:1, 0:512], lhsT=wlhs,
                        rhs=wlhs.broadcast_to([128, 512]),
                        start=True, stop=True, skip_group_check=True,
                    )

            # ---- phase 1: aggrT = x^T A^T + rel^T R^T (bf16 x fp8 A) ----
            apt = [xap.tile([128, 8, NPC], f8, tag=f"ap{j}", name=f"ap{j}")
                   for j in range(4)]
            xpt = [xap.tile([128, 8, 2, D], bf16, tag=f"xp{j}", name=f"xp{j}")
                   for j in range(2)]
            nc.sync.dma_start(xpt[0], xp_r[:, 0:8, :, :])
            nc.sync.dma_start(apt[0], ap_r[:, 0:8, :])
            nc.sync.dma_start(xpt[1], xp_r[:, 8:16, :, :])
            for j in range(1, 4):
                nc.sync.dma_start(apt[j], ap_r[:, 8 * j: 8 * j + 8, :])
            rpt = xap.tile([128, cfg.RPAD // 256, 2, D], bf16, tag="rpt")
            nc.sync.dma_start(rpt, rp_r)
            rrt = rap.tile([128, 8, NPC], f8, tag="rrt")
            nc.sync.dma_start(rrt, rr_r)

            aggr_ps = psAG.tile([128, NPC], f32, tag="ps_ag")
            for k in range(cfg.NKX):
                nc.tensor.matmul(
                    aggr_ps, lhsT=xpt[k // 16][:, (k % 16) // 2, k % 2, :],
                    rhs=apt[k // 8][:, k % 8, :],
                    start=(k == 0), stop=False, skip_group_check=True,
                )
            for k in range(cfg.NKR):
                nc.tensor.matmul(
                    aggr_ps, lhsT=rpt[:, k // 2, k % 2, :], rhs=rrt[:, k, :],
                    start=False, stop=(k == cfg.NKR - 1), skip_group_check=True,
                )

            # ---- one-time prep: aggr/4 as fp8 hi+lo and f32r ------------
            # (A/R are host-prescaled by 1/4, so aggr_ps is already aggr/4)
            a8p = prep.tile([128, 2, NPC], f8, tag="a8p")
            nc.scalar.activation(a8p[:, 0, :], aggr_ps, Copy)     # hi (ACT)
            nc.vector.tensor_tensor(a8p[:, 1, :], aggr_ps, a8p[:, 0, :],
                                    mybir.AluOpType.subtract)     # lo (DVE)
            a8r = prep.tile([128, NPC], f32r, tag="a8r")
            nc.scalar.activation(a8r, aggr_ps, Copy)              # f32r (ACT)

            # ---- main loop over entity super-chunks ---------------------
            # Software-pipelined: MM2(g-LAG) is emitted after MM1(g) so the
            # in-order PE never parks on a pending relu copy while the next
            # chunk's MM1 (the other engine's copy input) is still unissued.
            G_ps = psAG.tile([128, 128], f32, tag="ps_ag", name="G_ps")
            outT_ps = psO.tile([128, NPC], f32, tag="outps")
            LAG = lag
            pend = []  # (g, m2_tile, esb_tile, s)

            def mm2_drain(g2, m22, esb2, s2):
                nc.tensor.matmul(
                    outT_ps, lhsT=esb2[:, s2, :, :], rhs=m22,
                    start=(g2 == 0), stop=False,
                    perf_mode=DR, skip_group_check=True,
                )

            for b in range(NBLK):
                BC = cfg.BSC * 256
                ent = etnp.tile([128, BC], f8, tag="etn")
                nc.sync.dma_start(ent, etn_t[:, b * BC:(b + 1) * BC])
                esb = eswp.tile([128, cfg.BSC, 2, 128], f8, tag="esw")
                nc.sync.dma_start(esb, esw_t[b])
                n_sc = min(cfg.BSC, NSC - b * cfg.BSC)
                s = 0
                while s < n_sc:
                    bb = min(batch, n_sc - s)
                    items = []
                    for t in range(bb):
                        g = b * cfg.BSC + s + t
                        # Gram accumulation on the esw pair stream
                        nc.tensor.matmul(
                            G_ps, lhsT=esb[:, s + t, :, :],
                            rhs=esb[:, s + t, :, :],
                            start=(g == 0), stop=(g == NSC - 1),
                            perf_mode=DR, skip_group_check=True,
                        )
                        # MM1: scoresT for 2x128 entities (negated weights)
                        sps = psS.tile([128, 1024], f32, tag="sps")
                        for h in range(2):
                            c0 = (s + t) * 256 + h * 128
                            nc.tensor.matmul(
                                sps[:, h * NPC:(h + 1) * NPC],
                                lhsT=ent[:, c0: c0 + 128]
                                .unsqueeze(1).broadcast_to([128, 2, 128]),
                                rhs=a8p, start=True, stop=True,
                                perf_mode=DR, skip_group_check=True,
                            )
                        items.append((g, s + t, sps))
                    # relu pair-copies PSUM -> SBUF fp8 (the critical resource)
                    for g, st, sps in items:
                        m2 = m2p.tile([128, 2, NPC], f8, tag="m2")
                        spsr = sps.rearrange("p (a b) -> p a b", a=2)
                        if (g % dve_mod) in tuple(range(0, 2 * dve_cnt, 2)):
                            nc.vector.tensor_relu(m2, spsr)
                        else:
                            nc.scalar.activation(m2, spsr, Relu)
                        pend.append((g, m2, esb, st))
                    while len(pend) > LAG:
                        mm2_drain(*pend.pop(0))
                    s += bb
            for item in pend:
                mm2_drain(*item)

            # ---- tail: Gram term + final scale --------------------------
            g_sb = prep.tile([128, 128], f32r, tag="gsb")
            nc.scalar.activation(g_sb, G_ps, Copy)
            nc.tensor.matmul(outT_ps, lhsT=g_sb, rhs=a8r,
                             start=False, stop=True, skip_group_check=True)
            fin = prep.tile([128, NPC], f32, tag="fin")
            nc.scalar.activation(fin, outT_ps, Copy, scale=4.0)
            nc.sync.dma_start(out_t, fin)

    nc.compile()
    return nc


# --------------------------------------------------------------------------
# general path (arbitrary scale/bias): bf16 relu kernel (previous baseline)
# --------------------------------------------------------------------------

def host_prep_general(cfg, x, edge_index, relation_id, neg_flag, rel_emb,
                      entity_emb, scale, bias):
    src = np.asarray(edge_index[0]).astype(np.int64)
    dst = np.asarray(edge_index[1]).astype(np.int64)
    rel = np.asarray(relation_id).astype(np.int64)
    neg = np.asarray(neg_flag).astype(np.int64)
    x = np.asarray(x, F32)
    rel_emb = np.asarray(rel_emb, F32)
    entity_emb = np.asarray(entity_emb, F32)
    scale = np.asarray(scale, F32)
    bias = np.asarray(bias, F32)

    C, NPC, D = cfg.C, cfg.NPC, cfg.D
    negc = (1.0 - 2.0 * neg).astype(F32)

    A = np.zeros((cfg.N, cfg.N), F32)
    np.add.at(A, (dst, src), negc)
    A[np.arange(cfg.N), np.arange(cfg.N)] += 0.1
    Rm = np.zeros((cfg.N, cfg.RPAD), F32)
    np.add.at(Rm, (dst, rel), negc)

    vpad = cfg.VPAD_G
    E_pad = np.zeros((vpad, D), F32)
    E_pad[: cfg.V] = entity_emb
    et_tab = np.ascontiguousarray(E_pad.T).astype(BF16)
    e_sw = np.ascontiguousarray(
        E_pad.reshape(vpad // 512, 4, 128, D).transpose(0, 2, 1, 3)
    ).astype(BF16)
    scale_pad = np.ones(vpad, F32)
    scale_pad[: cfg.V] = scale
    bias_pad = np.zeros(vpad, F32)
    bias_pad[: cfg.V] = bias
    scaleT = np.ascontiguousarray(scale_pad.reshape(cfg.NV_G, 128).T)
    biasT = np.ascontiguousarray(bias_pad.reshape(cfg.NV_G, 128).T)

    xb = x.astype(BF16)
    rb = np.zeros((cfg.RPAD, D), F32)
    rb[: cfg.R] = rel_emb
    rb = rb.astype(BF16)

    shared = {"x_b": xb, "rel_b": rb, "et_tab": et_tab, "e_sw": e_sw,
              "scaleT": scaleT, "biasT": biasT}
    in_maps = []
    for c in range(C):
        rows = slice(c * NPC, (c + 1) * NPC)
        m = dict(shared)
        m.update({
            "a_t": np.ascontiguousarray(A[rows].T).astype(BF16),
            "r_t": np.ascontiguousarray(Rm[rows].T).astype(BF16),
        })
        in_maps.append(m)
    return in_maps


def build_general(cfg):
    f32, bf16 = mybir.dt.float32, mybir.dt.bfloat16
    nc = bacc.Bacc("TRN2", target_bir_lowering=False, debug=False,
                   enable_asserts=False)
    D, NPC, NV = cfg.D, cfg.NPC, cfg.NV_G

    xb_t = nc.dram_tensor("x_b", [cfg.N, D], bf16, kind="ExternalInput").ap()
    rb_t = nc.dram_tensor("rel_b", [cfg.RPAD, D], bf16, kind="ExternalInput").ap()
    at_t = nc.dram_tensor("a_t", [cfg.N, NPC], bf16, kind="ExternalInput").ap()
    rt_t = nc.dram_tensor("r_t", [cfg.RPAD, NPC], bf16, kind="ExternalInput").ap()
    ett_t = nc.dram_tensor("et_tab", [128, cfg.VPAD_G], bf16, kind="ExternalInput").ap()
    esw_t = nc.dram_tensor("e_sw", [cfg.VPAD_G // 512, 128, 4, D], bf16, kind="ExternalInput").ap()
    scl_t = nc.dram_tensor("scaleT", [128, NV], f32, kind="ExternalInput").ap()
    bia_t = nc.dram_tensor("biasT", [128, NV], f32, kind="ExternalInput").ap()
    out_t = nc.dram_tensor("out", [128, NPC], f32, kind="ExternalOutput").ap()

    Relu = mybir.ActivationFunctionType.Relu

    with tile.TileContext(nc) as tc:
        with (
            tc.tile_pool(name="const", bufs=1) as constp,
            tc.tile_pool(name="aggk", bufs=4) as akp,
            tc.tile_pool(name="etab", bufs=6) as ep,
            tc.tile_pool(name="scoresb", bufs=6) as scp,
            tc.tile_pool(name="psA", bufs=1, space="PSUM") as psA,
            tc.tile_pool(name="psS", bufs=6, space="PSUM") as psS,
            tc.tile_pool(name="psO", bufs=1, space="PSUM") as psO,
        ):
            sclt = constp.tile([128, NV], f32, tag="sc")
            nc.sync.dma_start(sclt, scl_t)
            biat = constp.tile([128, NV], f32, tag="bi")
            nc.sync.dma_start(biat, bia_t)
            aggrT_sb = constp.tile([128, NPC], bf16, tag="aggrT")
            out_sb = constp.tile([128, NPC], f32, tag="outsb")

            aggr_ps = psA.tile([128, NPC], f32, tag="aggrps")
            for k in range(cfg.NKX):
                ks = slice(k * 128, (k + 1) * 128)
                xk = akp.tile([128, D], bf16, tag="lhs")
                nc.sync.dma_start(xk, xb_t[ks, :])
                ak = akp.tile([128, NPC], bf16, tag="rhs")
                nc.sync.dma_start(ak, at_t[ks, :])
                nc.tensor.matmul(aggr_ps, lhsT=xk, rhs=ak,
                                 start=(k == 0), stop=False, skip_group_check=True)
            for k in range(cfg.NKR):
                ks = slice(k * 128, (k + 1) * 128)
                rk = akp.tile([128, D], bf16, tag="lhs")
                nc.sync.dma_start(rk, rb_t[ks, :])
                rrk = akp.tile([128, NPC], bf16, tag="rhs")
                nc.sync.dma_start(rrk, rt_t[ks, :])
                nc.tensor.matmul(aggr_ps, lhsT=rk, rhs=rrk,
                                 start=False, stop=(k == cfg.NKR - 1),
                                 skip_group_check=True)
            nc.vector.tensor_copy(aggrT_sb, aggr_ps)

            outT_ps = psO.tile([128, NPC], f32, tag="outps")
            for vb in range(cfg.VPAD_G // 512):
                ett = ep.tile([128, 512], bf16, tag="et")
                nc.sync.dma_start(ett, ett_t[:, vb * 512: (vb + 1) * 512])
                esw = ep.tile([128, 4, D], bf16, tag="ee")
                nc.sync.dma_start(esw, esw_t[vb])
                for j in range(4):
                    v = vb * 4 + j
                    sps = psS.tile([128, NPC], f32, tag="sps")
                    nc.tensor.matmul(sps, lhsT=ett[:, j * 128: (j + 1) * 128],
                                     rhs=aggrT_sb, start=True, stop=True,
                                     skip_group_check=True)
                    st_sb = scp.tile([128, NPC], bf16, tag="st")
                    nc.scalar.activation(st_sb, sps, Relu,
                                         bias=biat[:, v: v + 1],
                                         scale=sclt[:, v: v + 1])
                    nc.tensor.matmul(outT_ps, lhsT=esw[:, j, :], rhs=st_sb,
                                     start=(v == 0), stop=(v == NV - 1),
                                     skip_group_check=True)

            nc.vector.tensor_copy(out_sb, outT_ps)
            nc.sync.dma_start(out_t, out_sb)

    nc.compile()
    return nc


# --------------------------------------------------------------------------

def run(inputs, trace=False, cfg=None):
    global LAST_NC
    if cfg is None:
        cfg = Cfg()
    scale = np.asarray(inputs["scale"], F32)
    bias = np.asarray(inputs["bias"], F32)
    fast = bool(np.all(scale == 1.0) and np.all(bias == 0.0))
    if fast:
        in_maps = host_prep_fast(
            cfg, inputs["x"], inputs["edge_index"], inputs["relation_id"],
            inputs["neg_flag"], inputs["rel_emb"], inputs["entity_emb"])
        nc = build_fast(cfg)
    else:
        in_maps = host_prep_general(cfg, **{
            k: inputs[k] for k in
            ["x", "edge_index", "relation_id", "neg_flag", "rel_emb",
             "entity_emb", "scale", "bias"]})
        nc = build_general(cfg)
    LAST_NC = nc
    try:
        res = run_bass_kernel_spmd(
            nc, in_maps, core_ids=list(range(cfg.C)), trace=trace)
    except ModuleNotFoundError:
        res = run_bass_kernel_spmd(
            nc, in_maps, core_ids=list(range(cfg.C)), trace=False)
    outs = []
    for c in range(cfg.C):
        outs.append(np.ascontiguousarray(np.asarray(res.results[c]["out"]).T))
    full = np.concatenate(outs, axis=0).astype(np.float32)
    return full, res


def kernel(**inputs):
    full, _ = run(inputs, trace=False)
    return full


# revision 44
# speedup vs baseline: 1.6293x; 1.0044x over previous
"""Trainium2 Bass kernel for LMPNN-style GNN message passing + entity double-matmul.

Reference computation:
    msg      = (x[src] + rel_emb[rel]) * (1 - 2*neg)        # [E, D]
    aggr_out = segment_sum(msg, dst, N)                     # [N, D]
    aggr     = 0.1*x + aggr_out
    score    = relu((aggr @ E^T) * scale + bias)            # [N, V]
    out      = score @ E                                    # [N, D]

Strategy (8 NeuronCores, node-sharded, no collectives):
  * Core c owns nodes [c*512, (c+1)*512).
  * Message passing is re-expressed densely:  aggr = A @ x + R @ rel_emb,
    where A[n, m] = sum of (1-2*neg) over edges m->n (+0.1 on the diagonal)
    and R[n, r] the same per relation. Host builds the integer count
    matrices (index preprocessing only); the device does the FP work as
    bf16 TensorEngine matmuls accumulated in fp32 PSUM -> aggrT [D, 512].
  * Fast path (scale==1, bias==0): the double matmul runs in fp8-e4m3 with
    DoubleRow (2x PE throughput) using the identity
        relu(s) = s + relu(-s)
    =>  out = aggr @ G + relu(-s) @ E,   G = E8^T E8 (Gram, on device).
    The dominant coherent term aggr@G is computed exactly (f32r matmul),
    so e4m3 noise only touches the small fluctuation term relu(-s)@E and
    concentrates away over V -> measured rel err ~3.3e-3.
    - MM1: scoresT = (-E8^T dup-slots) x [a_hi|a_lo] pairs  (DoubleRow)
      with aggr/4 split hi/lo in e4m3 to keep the score error incoherent
      (A/R are host-prescaled by 1/4 so aggr_ps is already aggr/4).
    - relu copies PSUM->SBUF-fp8 alternate DVE/ACT (pair-granularity, two
      PSUM banks per op) -- these are the critical resource.
    - MM2: outT += E8-pairs x relu-pairs  (DoubleRow, 256 entities/inst).
    - MMG: G += E8-pairs x E8-pairs       (DoubleRow, on the same stream).
  * General scale/bias path falls back to the bf16 relu kernel.
  * Output is outT [128, 512] fp32 per core; host transposes/concats.
"""

import sys

import numpy as np

try:
    import concourse.bass as bass
except ImportError:  # pragma: no cover
    sys.path.insert(0, "/opt/trn_rl_repo")
    import concourse.bass as bass

import ml_dtypes

import concourse.bacc as bacc
import concourse.mybir as mybir
import concourse.tile as tile
from concourse.bass_utils import run_bass_kernel_spmd

BF16 = ml_dtypes.bfloat16
F8 = ml_dtypes.float8_e4m3
F32 = np.float32

LAST_NC = None


class Cfg:
    def __init__(self, N=4096, E=262144, D=128, R=1000, V=50000, C=8):
        self.N, self.E, self.D, self.R, self.V, self.C = N, E, D, R, V, C
        self.NPC = N // C                       # nodes per core
        assert self.NPC % 128 == 0 and N % 128 == 0
        self.RPAD = ((R + 127) // 128) * 128    # padded relation count
        self.VPAD = ((V + 255) // 256) * 256    # 256-entity super-chunks
        self.NSC = self.VPAD // 256             # super-chunk count (196)
        self.BSC = 16                           # super-chunks per DMA block
        bcols = self.BSC * 256
        self.NBLK = (self.VPAD + bcols - 1) // bcols
        self.VPAD2 = self.NBLK * bcols          # esw/etn padded cols
        self.NKX = N // 128                     # k-chunks for A @ x
        self.NKR = self.RPAD // 128             # k-chunks for R @ rel
        # old-path padding
        self.VPAD_G = ((V + 511) // 512) * 512
        self.NV_G = self.VPAD_G // 128


# --------------------------------------------------------------------------
# fast path: fp8 DoubleRow via relu(s) = s + relu(-s) and the Gram term
# --------------------------------------------------------------------------

def host_prep_fast(cfg, x, edge_index, relation_id, neg_flag, rel_emb,
                   entity_emb):
    src = np.asarray(edge_index[0]).astype(np.int64)
    dst = np.asarray(edge_index[1]).astype(np.int64)
    rel = np.asarray(relation_id).astype(np.int64)
    neg = np.asarray(neg_flag).astype(np.int64)
    x = np.asarray(x, F32)
    rel_emb = np.asarray(rel_emb, F32)
    entity_emb = np.asarray(entity_emb, F32)

    C, NPC, D = cfg.C, cfg.NPC, cfg.D
    negc = (1.0 - 2.0 * neg).astype(F32)

    # dense message-passing operators (index preprocessing). Pre-scaled by
    # 1/4 so the device-side aggregate is already aggr/4 (the fp8 score
    # range); quarter-integers are exact in e4m3, the 0.1 diagonal rounds
    # to 0.0254 (1.6% on a term that is ~1% of aggr).
    A = np.zeros((cfg.N, cfg.N), F32)
    np.add.at(A, (dst, src), negc)
    A[np.arange(cfg.N), np.arange(cfg.N)] += 0.1          # residual 0.1*x
    A *= 0.25
    Rm = np.zeros((cfg.N, cfg.RPAD), F32)
    np.add.at(Rm, (dst, rel), negc)
    Rm *= 0.25

    # fp8 entity table, padded
    Epad = np.zeros((cfg.VPAD2, D), F32)
    Epad[: cfg.V] = entity_emb
    E8 = Epad.astype(F8)                                   # e4m3 table
    E8f = E8.astype(F32)
    etn = np.ascontiguousarray((-E8f).T).astype(F8)        # [128, VPAD2], negated
    esw = np.ascontiguousarray(
        E8.reshape(cfg.NBLK, cfg.BSC, 2, 128, D).transpose(0, 3, 1, 2, 4)
    ).reshape(cfg.NBLK, 128, cfg.BSC * 256)                # [NBLK, 128, BSC*256]

    rb = np.zeros((cfg.RPAD, D), F32)
    rb[: cfg.R] = rel_emb

    # x/rel as e4m3 hi/lo pairs so phase 1 also runs DoubleRow: the slot dim
    # carries [hi|lo] (summing to the bf16-class value) while A^T/R^T ride
    # broadcast in both slots. Layout groups two k-chunks per row for 512B
    # innermost DMA runs: [k/2, 128, kk, slot, D].
    def hilo_pairs(t, nk):
        hi = t.astype(F8)
        lo = (t - hi.astype(F32)).astype(F8)
        pair = np.stack([hi, lo], axis=1)                  # [rows, 2, D]
        return np.ascontiguousarray(
            pair.reshape(nk // 2, 2, 128, 2, D).transpose(0, 2, 1, 3, 4))

    xp = hilo_pairs(x, cfg.N // 128)                       # [16,128,2,2,128]
    rp = hilo_pairs(rb, cfg.RPAD // 128)                   # [4,128,2,2,128]

    in_maps = []
    for c in range(C):
        rows = slice(c * NPC, (c + 1) * NPC)
        ap8 = np.ascontiguousarray(A[rows].T).astype(F8)   # [N, NPC]
        rr8 = np.ascontiguousarray(Rm[rows].T).astype(F8)  # [RPAD, NPC]
        in_maps.append({"xp": xp, "rp": rp, "ap8": ap8, "rr8": rr8,
                        "etn": etn, "esw": esw})
    return in_maps


def build_fast(cfg, lag=7, m2_bufs=None, dve_mod=15, dve_cnt=7, batch=1,
               warm=12, dve_pat=None):
    f32, bf16 = mybir.dt.float32, mybir.dt.bfloat16
    f8, f32r = mybir.dt.float8e4, mybir.dt.float32r
    DR = mybir.MatmulPerfMode.DoubleRow
    Relu = mybir.ActivationFunctionType.Relu
    Copy = mybir.ActivationFunctionType.Copy

    nc = bacc.Bacc("TRN2", target_bir_lowering=False, debug=False,
                   enable_asserts=False)
    D, NPC, NSC, NBLK = cfg.D, cfg.NPC, cfg.NSC, cfg.NBLK

    xp_t = nc.dram_tensor("xp", [cfg.N // 256, 128, 2, 2, D], f8, kind="ExternalInput").ap()
    rp_t = nc.dram_tensor("rp", [cfg.RPAD // 256, 128, 2, 2, D], f8, kind="ExternalInput").ap()
    ap_t = nc.dram_tensor("ap8", [cfg.N, NPC], f8, kind="ExternalInput").ap()
    rr_t = nc.dram_tensor("rr8", [cfg.RPAD, NPC], f8, kind="ExternalInput").ap()
    etn_t = nc.dram_tensor("etn", [128, cfg.VPAD2], f8, kind="ExternalInput").ap()
    esw_t = nc.dram_tensor("esw", [NBLK, 128, cfg.BSC * 256], f8, kind="ExternalInput").ap()
    out_t = nc.dram_tensor("out", [128, NPC], f32, kind="ExternalOutput").ap()

    xp_r = xp_t.rearrange("j p k s d -> p j k s d")        # [128, 16, 2, 2, 128]
    rp_r = rp_t.rearrange("j p k s d -> p j k s d")        # [128, 4, 2, 2, 128]
    ap_r = ap_t.rearrange("(k p) f -> p k f", p=128)       # [128, 32, 512]
    rr_r = rr_t.rearrange("(k p) f -> p k f", p=128)       # [128, 8, 512]

    with tile.TileContext(nc) as tc:
        with (
            tc.tile_pool(name="xap", bufs=4) as xap,
            tc.tile_pool(name="rap", bufs=1) as rap,
            tc.tile_pool(name="prep", bufs=1) as prep,
            tc.tile_pool(name="etnp", bufs=4) as etnp,
            tc.tile_pool(name="eswp", bufs=4) as eswp,
            tc.tile_pool(name="m2p", bufs=(m2_bufs or lag + 2)) as m2p,
            tc.tile_pool(name="psAG", bufs=1, space="PSUM") as psAG,
            tc.tile_pool(name="psS", bufs=3, space="PSUM") as psS,
            tc.tile_pool(name="psO", bufs=1, space="PSUM") as psO,
        ):
            # optional PE p-state warm-up: dependency-free matmuls on a
            # scratch const tile so phase-1 matmuls price at full clock
            if warm:
                wlhs = prep.tile([128, 1], bf16, tag="wlhs")
                nc.vector.memset(wlhs, 0.0)
                wps = psS.tile([128, 1024], f32, tag="sps", name="warm_ps")
                for i in range(warm):
                    nc.tensor.matmul(
                        wps[0:1, 0:512], lhsT=wlhs,
                        rhs=wlhs.broadcast_to([128, 512]),
                        start=True, stop=True, skip_group_check=True,
                    )

            # ---- phase 1: aggrT = x^T A^T + rel^T R^T (bf16 x fp8 A) ----
            apt = [xap.tile([128, 8, NPC], f8, tag=f"ap{j}", name=f"ap{j}")
                   for j in range(4)]
            xpt = [xap.tile([128, 8, 2, 2, D], f8, tag=f"xp{j}", name=f"xp{j}")
                   for j in range(2)]
            nc.sync.dma_start(xpt[0], xp_r[:, 0:8, :, :, :])
            nc.sync.dma_start(apt[0], ap_r[:, 0:8, :])
            nc.sync.dma_start(xpt[1], xp_r[:, 8:16, :, :, :])
            for j in range(1, 4):
                nc.sync.dma_start(apt[j], ap_r[:, 8 * j: 8 * j + 8, :])
            rpt = xap.tile([128, cfg.RPAD // 256, 2, 2, D], f8, tag="rpt")
            nc.sync.dma_start(rpt, rp_r)
            rrt = rap.tile([128, 8, NPC], f8, tag="rrt")
            nc.sync.dma_start(rrt, rr_r)

            aggr_ps = psAG.tile([128, NPC], f32, tag="ps_ag")
            for k in range(cfg.NKX):
                nc.tensor.matmul(
                    aggr_ps, lhsT=xpt[k // 16][:, (k % 16) // 2, k % 2, :, :],
                    rhs=apt[k // 8][:, k % 8, :]
                    .unsqueeze(1).broadcast_to([128, 2, NPC]),
                    start=(k == 0), stop=False,
                    perf_mode=DR, skip_group_check=True,
                )
            for k in range(cfg.NKR):
                nc.tensor.matmul(
                    aggr_ps, lhsT=rpt[:, k // 2, k % 2, :, :],
                    rhs=rrt[:, k, :].unsqueeze(1).broadcast_to([128, 2, NPC]),
                    start=False, stop=(k == cfg.NKR - 1),
                    perf_mode=DR, skip_group_check=True,
                )

            # ---- one-time prep: aggr/4 as fp8 hi+lo and f32r ------------
            # (A/R are host-prescaled by 1/4, so aggr_ps is already aggr/4)
            a8p = prep.tile([128, 2, NPC], f8, tag="a8p")
            nc.scalar.activation(a8p[:, 0, :], aggr_ps, Copy)     # hi (ACT)
            nc.vector.tensor_tensor(a8p[:, 1, :], aggr_ps, a8p[:, 0, :],
                                    mybir.AluOpType.subtract)     # lo (DVE)
            a8r = prep.tile([128, NPC], f32r, tag="a8r")
            nc.scalar.activation(a8r, aggr_ps, Copy)              # f32r (ACT)

            # ---- main loop over entity super-chunks ---------------------
            # Software-pipelined: MM2(g-LAG) is emitted after MM1(g) so the
            # in-order PE never parks on a pending relu copy while the next
            # chunk's MM1 (the other engine's copy input) is still unissued.
            G_ps = psAG.tile([128, 128], f32, tag="ps_ag", name="G_ps")
            outT_ps = psO.tile([128, NPC], f32, tag="outps")
            LAG = lag
            pend = []  # (g, m2_tile, esb_tile, s)

            def mm2_drain(g2, m22, esb2, s2):
                nc.tensor.matmul(
                    outT_ps, lhsT=esb2[:, s2, :, :], rhs=m22,
                    start=(g2 == 0), stop=False,
                    perf_mode=DR, skip_group_check=True,
                )

            for b in range(NBLK):
                BC = cfg.BSC * 256
                ent = etnp.tile([128, BC], f8, tag="etn")
                nc.sync.dma_start(ent, etn_t[:, b * BC:(b + 1) * BC])
                esb = eswp.tile([128, cfg.BSC, 2, 128], f8, tag="esw")
                nc.sync.dma_start(esb, esw_t[b])
                n_sc = min(cfg.BSC, NSC - b * cfg.BSC)
                s = 0
                while s < n_sc:
                    bb = min(batch, n_sc - s)
                    items = []
                    for t in range(bb):
                        g = b * cfg.BSC + s + t
                        # Gram accumulation on the esw pair stream
                        nc.tensor.matmul(
                            G_ps, lhsT=esb[:, s + t, :, :],
                            rhs=esb[:, s + t, :, :],
                            start=(g == 0), stop=(g == NSC - 1),
                            perf_mode=DR, skip_group_check=True,
                        )
                        # MM1: scoresT for 2x128 entities (negated weights)
                        sps = psS.tile([128, 1024], f32, tag="sps")
                        for h in range(2):
                            c0 = (s + t) * 256 + h * 128
                            nc.tensor.matmul(
                                sps[:, h * NPC:(h + 1) * NPC],
                                lhsT=ent[:, c0: c0 + 128]
                                .unsqueeze(1).broadcast_to([128, 2, 128]),
                                rhs=a8p, start=True, stop=True,
                                perf_mode=DR, skip_group_check=True,
                            )
                        items.append((g, s + t, sps))
                    # relu pair-copies PSUM -> SBUF fp8 (the critical resource)
                    for g, st, sps in items:
                        m2 = m2p.tile([128, 2, NPC], f8, tag="m2")
                        spsr = sps.rearrange("p (a b) -> p a b", a=2)
                        sel = (tuple(int(c) for c in str(dve_pat)) if dve_pat
                               else None)
                        on_dve = (sel[g % len(sel)] == 1 if sel else
                                  (g % dve_mod) in tuple(range(0, 2 * dve_cnt, 2)))
                        if on_dve:
                            nc.vector.tensor_relu(m2, spsr)
                        else:
                            nc.scalar.activation(m2, spsr, Relu)
                        pend.append((g, m2, esb, st))
                    while len(pend) > LAG:
                        mm2_drain(*pend.pop(0))
                    s += bb
            for item in pend:
                mm2_drain(*item)

            # ---- tail: Gram term + final scale --------------------------
            g_sb = prep.tile([128, 128], f32r, tag="gsb")
            nc.scalar.activation(g_sb, G_ps, Copy)
            nc.tensor.matmul(outT_ps, lhsT=g_sb, rhs=a8r,
                             start=False, stop=True, skip_group_check=True)
            fin = prep.tile([128, NPC], f32, tag="fin")
            nc.scalar.activation(fin, outT_ps, Copy, scale=4.0)
            nc.sync.dma_start(out_t, fin)

    nc.compile()
    return nc


# --------------------------------------------------------------------------
# general path (arbitrary scale/bias): bf16 relu kernel (previous baseline)
# --------------------------------------------------------------------------

def host_prep_general(cfg, x, edge_index, relation_id, neg_flag, rel_emb,
                      entity_emb, scale, bias):
    src = np.asarray(edge_index[0]).astype(np.int64)
    dst = np.asarray(edge_index[1]).astype(np.int64)
    rel = np.asarray(relation_id).astype(np.int64)
    neg = np.asarray(neg_flag).astype(np.int64)
    x = np.asarray(x, F32)
    rel_emb = np.asarray(rel_emb, F32)
    entity_emb = np.asarray(entity_emb, F32)
    scale = np.asarray(scale, F32)
    bias = np.asarray(bias, F32)

    C, NPC, D = cfg.C, cfg.NPC, cfg.D
    negc = (1.0 - 2.0 * neg).astype(F32)

    A = np.zeros((cfg.N, cfg.N), F32)
    np.add.at(A, (dst, src), negc)
    A[np.arange(cfg.N), np.arange(cfg.N)] += 0.1
    Rm = np.zeros((cfg.N, cfg.RPAD), F32)
    np.add.at(Rm, (dst, rel), negc)

    vpad = cfg.VPAD_G
    E_pad = np.zeros((vpad, D), F32)
    E_pad[: cfg.V] = entity_emb
    et_tab = np.ascontiguousarray(E_pad.T).astype(BF16)
    e_sw = np.ascontiguousarray(
        E_pad.reshape(vpad // 512, 4, 128, D).transpose(0, 2, 1, 3)
    ).astype(BF16)
    scale_pad = np.ones(vpad, F32)
    scale_pad[: cfg.V] = scale
    bias_pad = np.zeros(vpad, F32)
    bias_pad[: cfg.V] = bias
    scaleT = np.ascontiguousarray(scale_pad.reshape(cfg.NV_G, 128).T)
    biasT = np.ascontiguousarray(bias_pad.reshape(cfg.NV_G, 128).T)

    xb = x.astype(BF16)
    rb = np.zeros((cfg.RPAD, D), F32)
    rb[: cfg.R] = rel_emb
    rb = rb.astype(BF16)

    shared = {"x_b": xb, "rel_b": rb, "et_tab": et_tab, "e_sw": e_sw,
              "scaleT": scaleT, "biasT": biasT}
    in_maps = []
    for c in range(C):
        rows = slice(c * NPC, (c + 1) * NPC)
        m = dict(shared)
        m.update({
            "a_t": np.ascontiguousarray(A[rows].T).astype(BF16),
            "r_t": np.ascontiguousarray(Rm[rows].T).astype(BF16),
        })
        in_maps.append(m)
    return in_maps


def build_general(cfg):
    f32, bf16 = mybir.dt.float32, mybir.dt.bfloat16
    nc = bacc.Bacc("TRN2", target_bir_lowering=False, debug=False,
                   enable_asserts=False)
    D, NPC, NV = cfg.D, cfg.NPC, cfg.NV_G

    xb_t = nc.dram_tensor("x_b", [cfg.N, D], bf16, kind="ExternalInput").ap()
    rb_t = nc.dram_tensor("rel_b", [cfg.RPAD, D], bf16, kind="ExternalInput").ap()
    at_t = nc.dram_tensor("a_t", [cfg.N, NPC], bf16, kind="ExternalInput").ap()
    rt_t = nc.dram_tensor("r_t", [cfg.RPAD, NPC], bf16, kind="ExternalInput").ap()
    ett_t = nc.dram_tensor("et_tab", [128, cfg.VPAD_G], bf16, kind="ExternalInput").ap()
    esw_t = nc.dram_tensor("e_sw", [cfg.VPAD_G // 512, 128, 4, D], bf16, kind="ExternalInput").ap()
    scl_t = nc.dram_tensor("scaleT", [128, NV], f32, kind="ExternalInput").ap()
    bia_t = nc.dram_tensor("biasT", [128, NV], f32, kind="ExternalInput").ap()
    out_t = nc.dram_tensor("out", [128, NPC], f32, kind="ExternalOutput").ap()

    Relu = mybir.ActivationFunctionType.Relu

    with tile.TileContext(nc) as tc:
        with (
            tc.tile_pool(name="const", bufs=1) as constp,
            tc.tile_pool(name="aggk", bufs=4) as akp,
            tc.tile_pool(name="etab", bufs=6) as ep,
            tc.tile_pool(name="scoresb", bufs=6) as scp,
            tc.tile_pool(name="psA", bufs=1, space="PSUM") as psA,
            tc.tile_pool(name="psS", bufs=6, space="PSUM") as psS,
            tc.tile_pool(name="psO", bufs=1, space="PSUM") as psO,
        ):
            sclt = constp.tile([128, NV], f32, tag="sc")
            nc.sync.dma_start(sclt, scl_t)
            biat = constp.tile([128, NV], f32, tag="bi")
            nc.sync.dma_start(biat, bia_t)
            aggrT_sb = constp.tile([128, NPC], bf16, tag="aggrT")
            out_sb = constp.tile([128, NPC], f32, tag="outsb")

            aggr_ps = psA.tile([128, NPC], f32, tag="aggrps")
            for k in range(cfg.NKX):
                ks = slice(k * 128, (k + 1) * 128)
                xk = akp.tile([128, D], bf16, tag="lhs")
                nc.sync.dma_start(xk, xb_t[ks, :])
                ak = akp.tile([128, NPC], bf16, tag="rhs")
                nc.sync.dma_start(ak, at_t[ks, :])
                nc.tensor.matmul(aggr_ps, lhsT=xk, rhs=ak,
                                 start=(k == 0), stop=False, skip_group_check=True)
            for k in range(cfg.NKR):
                ks = slice(k * 128, (k + 1) * 128)
                rk = akp.tile([128, D], bf16, tag="lhs")
                nc.sync.dma_start(rk, rb_t[ks, :])
                rrk = akp.tile([128, NPC], bf16, tag="rhs")
                nc.sync.dma_start(rrk, rt_t[ks, :])
                nc.tensor.matmul(aggr_ps, lhsT=rk, rhs=rrk,
                                 start=False, stop=(k == cfg.NKR - 1),
                                 skip_group_check=True)
            nc.vector.tensor_copy(aggrT_sb, aggr_ps)

            outT_ps = psO.tile([128, NPC], f32, tag="outps")
            for vb in range(cfg.VPAD_G // 512):
                ett = ep.tile([128, 512], bf16, tag="et")
                nc.sync.dma_start(ett, ett_t[:, vb * 512: (vb + 1) * 512])
                esw = ep.tile([128, 4, D], bf16, tag="ee")
                nc.sync.dma_start(esw, esw_t[vb])
                for j in range(4):
                    v = vb * 4 + j
                    sps = psS.tile([128, NPC], f32, tag="sps")
                    nc.tensor.matmul(sps, lhsT=ett[:, j * 128: (j + 1) * 128],
                                     rhs=aggrT_sb, start=True, stop=True,
                                     skip_group_check=True)
                    st_sb = scp.tile([128, NPC], bf16, tag="st")
                    nc.scalar.activation(st_sb, sps, Relu,
                                         bias=biat[:, v: v + 1],
                                         scale=sclt[:, v: v + 1])
                    nc.tensor.matmul(outT_ps, lhsT=esw[:, j, :], rhs=st_sb,
                                     start=(v == 0), stop=(v == NV - 1),
                                     skip_group_check=True)

            nc.vector.tensor_copy(out_sb, outT_ps)
            nc.sync.dma_start(out_t, out_sb)

    nc.compile()
    return nc


# --------------------------------------------------------------------------

def run(inputs, trace=False, cfg=None):
    global LAST_NC
    if cfg is None:
        cfg = Cfg()
    scale = np.asarray(inputs["scale"], F32)
    bias = np.asarray(inputs["bias"], F32)
    fast = bool(np.all(scale == 1.0) and np.all(bias == 0.0))
    if fast:
        in_maps = host_prep_fast(
            cfg, inputs["x"], inputs["edge_index"], inputs["relation_id"],
            inputs["neg_flag"], inputs["rel_emb"], inputs["entity_emb"])
        nc = build_fast(cfg)
    else:
        in_maps = host_prep_general(cfg, **{
            k: inputs[k] for k in
            ["x", "edge_index", "relation_id", "neg_flag", "rel_emb",
             "entity_emb", "scale", "bias"]})
        nc = build_general(cfg)
    LAST_NC = nc
    try:
        res = run_bass_kernel_spmd(
            nc, in_maps, core_ids=list(range(cfg.C)), trace=trace)
    except ModuleNotFoundError:
        res = run_bass_kernel_spmd(
            nc, in_maps, core_ids=list(range(cfg.C)), trace=False)
    outs = []
    for c in range(cfg.C):
        outs.append(np.ascontiguousarray(np.asarray(res.results[c]["out"]).T))
    full = np.concatenate(outs, axis=0).astype(np.float32)
    return full, res


def kernel(**inputs):
    full, _ = run(inputs, trace=False)
    return full


# revision 47
# speedup vs baseline: 1.6306x; 1.0008x over previous
"""Trainium2 Bass kernel for LMPNN-style GNN message passing + entity double-matmul.

Reference computation:
    msg      = (x[src] + rel_emb[rel]) * (1 - 2*neg)        # [E, D]
    aggr_out = segment_sum(msg, dst, N)                     # [N, D]
    aggr     = 0.1*x + aggr_out
    score    = relu((aggr @ E^T) * scale + bias)            # [N, V]
    out      = score @ E                                    # [N, D]

Strategy (8 NeuronCores, node-sharded, no collectives):
  * Core c owns nodes [c*512, (c+1)*512).
  * Message passing is re-expressed densely:  aggr = A @ x + R @ rel_emb,
    where A[n, m] = sum of (1-2*neg) over edges m->n (+0.1 on the diagonal)
    and R[n, r] the same per relation. Host builds the integer count
    matrices (index preprocessing only); the device does the FP work as
    bf16 TensorEngine matmuls accumulated in fp32 PSUM -> aggrT [D, 512].
  * Fast path (scale==1, bias==0): the double matmul runs in fp8-e4m3 with
    DoubleRow (2x PE throughput) using the identity
        relu(s) = s + relu(-s)
    =>  out = aggr @ G + relu(-s) @ E,   G = E8^T E8 (Gram, on device).
    The dominant coherent term aggr@G is computed exactly (f32r matmul),
    so e4m3 noise only touches the small fluctuation term relu(-s)@E and
    concentrates away over V -> measured rel err ~3.3e-3.
    - MM1: scoresT = (-E8^T dup-slots) x [a_hi|a_lo] pairs  (DoubleRow)
      with aggr/4 split hi/lo in e4m3 to keep the score error incoherent
      (A/R are host-prescaled by 1/4 so aggr_ps is already aggr/4).
    - relu copies PSUM->SBUF-fp8 alternate DVE/ACT (pair-granularity, two
      PSUM banks per op) -- these are the critical resource.
    - MM2: outT += E8-pairs x relu-pairs  (DoubleRow, 256 entities/inst).
    - MMG: G += E8-pairs x E8-pairs       (DoubleRow, on the same stream).
  * General scale/bias path falls back to the bf16 relu kernel.
  * Output is outT [128, 512] fp32 per core; host transposes/concats.
"""

import sys

import numpy as np

try:
    import concourse.bass as bass
except ImportError:  # pragma: no cover
    sys.path.insert(0, "/opt/trn_rl_repo")
    import concourse.bass as bass

import ml_dtypes

import concourse.bacc as bacc
import concourse.mybir as mybir
import concourse.tile as tile
from concourse.bass_utils import run_bass_kernel_spmd

BF16 = ml_dtypes.bfloat16
F8 = ml_dtypes.float8_e4m3
F32 = np.float32

LAST_NC = None


class Cfg:
    def __init__(self, N=4096, E=262144, D=128, R=1000, V=50000, C=8):
        self.N, self.E, self.D, self.R, self.V, self.C = N, E, D, R, V, C
        self.NPC = N // C                       # nodes per core
        assert self.NPC % 128 == 0 and N % 128 == 0
        self.RPAD = ((R + 127) // 128) * 128    # padded relation count
        self.VPAD = ((V + 255) // 256) * 256    # 256-entity super-chunks
        self.NSC = self.VPAD // 256             # super-chunk count (196)
        self.BSC = 16                           # super-chunks per DMA block
        bcols = self.BSC * 256
        self.NBLK = (self.VPAD + bcols - 1) // bcols
        self.VPAD2 = self.NBLK * bcols          # esw/etn padded cols
        self.NKX = N // 128                     # k-chunks for A @ x
        self.NKR = self.RPAD // 128             # k-chunks for R @ rel
        # old-path padding
        self.VPAD_G = ((V + 511) // 512) * 512
        self.NV_G = self.VPAD_G // 128


# --------------------------------------------------------------------------
# fast path: fp8 DoubleRow via relu(s) = s + relu(-s) and the Gram term
# --------------------------------------------------------------------------

def host_prep_fast(cfg, x, edge_index, relation_id, neg_flag, rel_emb,
                   entity_emb):
    src = np.asarray(edge_index[0]).astype(np.int64)
    dst = np.asarray(edge_index[1]).astype(np.int64)
    rel = np.asarray(relation_id).astype(np.int64)
    neg = np.asarray(neg_flag).astype(np.int64)
    x = np.asarray(x, F32)
    rel_emb = np.asarray(rel_emb, F32)
    entity_emb = np.asarray(entity_emb, F32)

    C, NPC, D = cfg.C, cfg.NPC, cfg.D
    negc = (1.0 - 2.0 * neg).astype(F32)

    # dense message-passing operators (index preprocessing). Pre-scaled by
    # 1/4 so the device-side aggregate is already aggr/4 (the fp8 score
    # range); quarter-integers are exact in e4m3, the 0.1 diagonal rounds
    # to 0.0254 (1.6% on a term that is ~1% of aggr).
    A = np.zeros((cfg.N, cfg.N), F32)
    np.add.at(A, (dst, src), negc)
    A[np.arange(cfg.N), np.arange(cfg.N)] += 0.1          # residual 0.1*x
    A *= 0.25
    Rm = np.zeros((cfg.N, cfg.RPAD), F32)
    np.add.at(Rm, (dst, rel), negc)
    Rm *= 0.25

    # fp8 entity table, padded
    Epad = np.zeros((cfg.VPAD2, D), F32)
    Epad[: cfg.V] = entity_emb
    E8 = Epad.astype(F8)                                   # e4m3 table
    E8f = E8.astype(F32)
    etn = np.ascontiguousarray((-E8f).T).astype(F8)        # [128, VPAD2], negated
    esw = np.ascontiguousarray(
        E8.reshape(cfg.NBLK, cfg.BSC, 2, 128, D).transpose(0, 3, 1, 2, 4)
    ).reshape(cfg.NBLK, 128, cfg.BSC * 256)                # [NBLK, 128, BSC*256]

    rb = np.zeros((cfg.RPAD, D), F32)
    rb[: cfg.R] = rel_emb

    # x/rel as e4m3 hi/lo pairs so phase 1 also runs DoubleRow: the slot dim
    # carries [hi|lo] (summing to the bf16-class value) while A^T/R^T ride
    # broadcast in both slots. Layout groups two k-chunks per row for 512B
    # innermost DMA runs: [k/2, 128, kk, slot, D].
    def hilo_pairs(t, nk):
        hi = t.astype(F8)
        lo = (t - hi.astype(F32)).astype(F8)
        pair = np.stack([hi, lo], axis=1)                  # [rows, 2, D]
        return np.ascontiguousarray(
            pair.reshape(nk // 2, 2, 128, 2, D).transpose(0, 2, 1, 3, 4))

    xp = hilo_pairs(x, cfg.N // 128)                       # [16,128,2,2,128]
    rp = hilo_pairs(rb, cfg.RPAD // 128)                   # [4,128,2,2,128]

    in_maps = []
    for c in range(C):
        rows = slice(c * NPC, (c + 1) * NPC)
        ap8 = np.ascontiguousarray(A[rows].T).astype(F8)   # [N, NPC]
        rr8 = np.ascontiguousarray(Rm[rows].T).astype(F8)  # [RPAD, NPC]
        in_maps.append({"xp": xp, "rp": rp, "ap8": ap8, "rr8": rr8,
                        "etn": etn, "esw": esw})
    return in_maps


def build_fast(cfg, lag=9, m2_bufs=None, dve_mod=17, dve_cnt=8, batch=1,
               warm=12, dve_pat=None):
    f32, bf16 = mybir.dt.float32, mybir.dt.bfloat16
    f8, f32r = mybir.dt.float8e4, mybir.dt.float32r
    DR = mybir.MatmulPerfMode.DoubleRow
    Relu = mybir.ActivationFunctionType.Relu
    Copy = mybir.ActivationFunctionType.Copy

    nc = bacc.Bacc("TRN2", target_bir_lowering=False, debug=False,
                   enable_asserts=False)
    D, NPC, NSC, NBLK = cfg.D, cfg.NPC, cfg.NSC, cfg.NBLK

    xp_t = nc.dram_tensor("xp", [cfg.N // 256, 128, 2, 2, D], f8, kind="ExternalInput").ap()
    rp_t = nc.dram_tensor("rp", [cfg.RPAD // 256, 128, 2, 2, D], f8, kind="ExternalInput").ap()
    ap_t = nc.dram_tensor("ap8", [cfg.N, NPC], f8, kind="ExternalInput").ap()
    rr_t = nc.dram_tensor("rr8", [cfg.RPAD, NPC], f8, kind="ExternalInput").ap()
    etn_t = nc.dram_tensor("etn", [128, cfg.VPAD2], f8, kind="ExternalInput").ap()
    esw_t = nc.dram_tensor("esw", [NBLK, 128, cfg.BSC * 256], f8, kind="ExternalInput").ap()
    out_t = nc.dram_tensor("out", [128, NPC], f32, kind="ExternalOutput").ap()

    xp_r = xp_t.rearrange("j p k s d -> p j k s d")        # [128, 16, 2, 2, 128]
    rp_r = rp_t.rearrange("j p k s d -> p j k s d")        # [128, 4, 2, 2, 128]
    ap_r = ap_t.rearrange("(k p) f -> p k f", p=128)       # [128, 32, 512]
    rr_r = rr_t.rearrange("(k p) f -> p k f", p=128)       # [128, 8, 512]

    with tile.TileContext(nc) as tc:
        with (
            tc.tile_pool(name="xap", bufs=4) as xap,
            tc.tile_pool(name="rap", bufs=1) as rap,
            tc.tile_pool(name="prep", bufs=1) as prep,
            tc.tile_pool(name="etnp", bufs=4) as etnp,
            tc.tile_pool(name="eswp", bufs=4) as eswp,
            tc.tile_pool(name="m2p", bufs=(m2_bufs or lag + 2)) as m2p,
            tc.tile_pool(name="psAG", bufs=1, space="PSUM") as psAG,
            tc.tile_pool(name="psS", bufs=3, space="PSUM") as psS,
            tc.tile_pool(name="psO", bufs=1, space="PSUM") as psO,
        ):
            # optional PE p-state warm-up: dependency-free matmuls on a
            # scratch const tile so phase-1 matmuls price at full clock
            if warm:
                wlhs = prep.tile([128, 1], bf16, tag="wlhs")
                nc.vector.memset(wlhs, 0.0)
                wps = psS.tile([128, 1024], f32, tag="sps", name="warm_ps")
                for i in range(warm):
                    nc.tensor.matmul(
                        wps[0:1, 0:512], lhsT=wlhs,
                        rhs=wlhs.broadcast_to([128, 512]),
                        start=True, stop=True, skip_group_check=True,
                    )

            # ---- phase 1: aggrT = x^T A^T + rel^T R^T (bf16 x fp8 A) ----
            apt = [xap.tile([128, 8, NPC], f8, tag=f"ap{j}", name=f"ap{j}")
                   for j in range(4)]
            xpt = [xap.tile([128, 8, 2, 2, D], f8, tag=f"xp{j}", name=f"xp{j}")
                   for j in range(2)]
            nc.sync.dma_start(xpt[0], xp_r[:, 0:8, :, :, :])
            nc.sync.dma_start(apt[0], ap_r[:, 0:8, :])
            nc.sync.dma_start(xpt[1], xp_r[:, 8:16, :, :, :])
            for j in range(1, 4):
                nc.sync.dma_start(apt[j], ap_r[:, 8 * j: 8 * j + 8, :])
            rpt = xap.tile([128, cfg.RPAD // 256, 2, 2, D], f8, tag="rpt")
            nc.sync.dma_start(rpt, rp_r)
            rrt = rap.tile([128, 8, NPC], f8, tag="rrt")
            nc.sync.dma_start(rrt, rr_r)

            aggr_ps = psAG.tile([128, NPC], f32, tag="ps_ag")
            for k in range(cfg.NKX):
                nc.tensor.matmul(
                    aggr_ps, lhsT=xpt[k // 16][:, (k % 16) // 2, k % 2, :, :],
                    rhs=apt[k // 8][:, k % 8, :]
                    .unsqueeze(1).broadcast_to([128, 2, NPC]),
                    start=(k == 0), stop=False,
                    perf_mode=DR, skip_group_check=True,
                )
            for k in range(cfg.NKR):
                nc.tensor.matmul(
                    aggr_ps, lhsT=rpt[:, k // 2, k % 2, :, :],
                    rhs=rrt[:, k, :].unsqueeze(1).broadcast_to([128, 2, NPC]),
                    start=False, stop=(k == cfg.NKR - 1),
                    perf_mode=DR, skip_group_check=True,
                )

            # ---- one-time prep: aggr/4 as fp8 hi+lo and f32r ------------
            # (A/R are host-prescaled by 1/4, so aggr_ps is already aggr/4)
            a8p = prep.tile([128, 2, NPC], f8, tag="a8p")
            nc.scalar.activation(a8p[:, 0, :], aggr_ps, Copy)     # hi (ACT)
            nc.vector.tensor_tensor(a8p[:, 1, :], aggr_ps, a8p[:, 0, :],
                                    mybir.AluOpType.subtract)     # lo (DVE)
            a8r = prep.tile([128, NPC], f32r, tag="a8r")
            nc.scalar.activation(a8r, aggr_ps, Copy)              # f32r (ACT)

            # ---- main loop over entity super-chunks ---------------------
            # Software-pipelined: MM2(g-LAG) is emitted after MM1(g) so the
            # in-order PE never parks on a pending relu copy while the next
            # chunk's MM1 (the other engine's copy input) is still unissued.
            G_ps = psAG.tile([128, 128], f32, tag="ps_ag", name="G_ps")
            outT_ps = psO.tile([128, NPC], f32, tag="outps")
            LAG = lag
            pend = []  # (g, m2_tile, esb_tile, s)

            def mm2_drain(g2, m22, esb2, s2):
                nc.tensor.matmul(
                    outT_ps, lhsT=esb2[:, s2, :, :], rhs=m22,
                    start=(g2 == 0), stop=False,
                    perf_mode=DR, skip_group_check=True,
                )

            for b in range(NBLK):
                BC = cfg.BSC * 256
                ent = etnp.tile([128, BC], f8, tag="etn")
                nc.sync.dma_start(ent, etn_t[:, b * BC:(b + 1) * BC])
                esb = eswp.tile([128, cfg.BSC, 2, 128], f8, tag="esw")
                nc.sync.dma_start(esb, esw_t[b])
                n_sc = min(cfg.BSC, NSC - b * cfg.BSC)
                s = 0
                while s < n_sc:
                    bb = min(batch, n_sc - s)
                    items = []
                    for t in range(bb):
                        g = b * cfg.BSC + s + t
                        # Gram accumulation on the esw pair stream
                        nc.tensor.matmul(
                            G_ps, lhsT=esb[:, s + t, :, :],
                            rhs=esb[:, s + t, :, :],
                            start=(g == 0), stop=(g == NSC - 1),
                            perf_mode=DR, skip_group_check=True,
                        )
                        # MM1: scoresT for 2x128 entities (negated weights)
                        sps = psS.tile([128, 1024], f32, tag="sps")
                        for h in range(2):
                            c0 = (s + t) * 256 + h * 128
                            nc.tensor.matmul(
                                sps[:, h * NPC:(h + 1) * NPC],
                                lhsT=ent[:, c0: c0 + 128]
                                .unsqueeze(1).broadcast_to([128, 2, 128]),
                                rhs=a8p, start=True, stop=True,
                                perf_mode=DR, skip_group_check=True,
                            )
                        items.append((g, s + t, sps))
                    # relu pair-copies PSUM -> SBUF fp8 (the critical resource)
                    for g, st, sps in items:
                        m2 = m2p.tile([128, 2, NPC], f8, tag="m2")
                        spsr = sps.rearrange("p (a b) -> p a b", a=2)
                        sel = (tuple(int(c) for c in str(dve_pat)) if dve_pat
                               else None)
                        on_dve = (sel[g % len(sel)] == 1 if sel else
                                  (g % dve_mod) in tuple(range(0, 2 * dve_cnt, 2)))
                        if on_dve:
                            nc.vector.tensor_relu(m2, spsr)
                        else:
                            nc.scalar.activation(m2, spsr, Relu)
                        pend.append((g, m2, esb, st))
                    while len(pend) > LAG:
                        mm2_drain(*pend.pop(0))
                    s += bb
            for item in pend:
                mm2_drain(*item)

            # ---- tail: Gram term + final scale --------------------------
            g_sb = prep.tile([128, 128], f32r, tag="gsb")
            nc.scalar.activation(g_sb, G_ps, Copy)
            nc.tensor.matmul(outT_ps, lhsT=g_sb, rhs=a8r,
                             start=False, stop=True, skip_group_check=True)
            fin = prep.tile([128, NPC], f32, tag="fin")
            nc.scalar.activation(fin, outT_ps, Copy, scale=4.0)
            nc.sync.dma_start(out_t, fin)

    nc.compile()
    return nc


# --------------------------------------------------------------------------
# general path (arbitrary scale/bias): bf16 relu kernel (previous baseline)
# --------------------------------------------------------------------------

def host_prep_general(cfg, x, edge_index, relation_id, neg_flag, rel_emb,
                      entity_emb, scale, bias):
    src = np.asarray(edge_index[0]).astype(np.int64)
    dst = np.asarray(edge_index[1]).astype(np.int64)
    rel = np.asarray(relation_id).astype(np.int64)
    neg = np.asarray(neg_flag).astype(np.int64)
    x = np.asarray(x, F32)
    rel_emb = np.asarray(rel_emb, F32)
    entity_emb = np.asarray(entity_emb, F32)
    scale = np.asarray(scale, F32)
    bias = np.asarray(bias, F32)

    C, NPC, D = cfg.C, cfg.NPC, cfg.D
    negc = (1.0 - 2.0 * neg).astype(F32)

    A = np.zeros((cfg.N, cfg.N), F32)
    np.add.at(A, (dst, src), negc)
    A[np.arange(cfg.N), np.arange(cfg.N)] += 0.1
    Rm = np.zeros((cfg.N, cfg.RPAD), F32)
    np.add.at(Rm, (dst, rel), negc)

    vpad = cfg.VPAD_G
    E_pad = np.zeros((vpad, D), F32)
    E_pad[: cfg.V] = entity_emb
    et_tab = np.ascontiguousarray(E_pad.T).astype(BF16)
    e_sw = np.ascontiguousarray(
        E_pad.reshape(vpad // 512, 4, 128, D).transpose(0, 2, 1, 3)
    ).astype(BF16)
    scale_pad = np.ones(vpad, F32)
    scale_pad[: cfg.V] = scale
    bias_pad = np.zeros(vpad, F32)
    bias_pad[: cfg.V] = bias
    scaleT = np.ascontiguousarray(scale_pad.reshape(cfg.NV_G, 128).T)
    biasT = np.ascontiguousarray(bias_pad.reshape(cfg.NV_G, 128).T)

    xb = x.astype(BF16)
    rb = np.zeros((cfg.RPAD, D), F32)
    rb[: cfg.R] = rel_emb
    rb = rb.astype(BF16)

    shared = {"x_b": xb, "rel_b": rb, "et_tab": et_tab, "e_sw": e_sw,
              "scaleT": scaleT, "biasT": biasT}
    in_maps = []
    for c in range(C):
        rows = slice(c * NPC, (c + 1) * NPC)
        m = dict(shared)
        m.update({
            "a_t": np.ascontiguousarray(A[rows].T).astype(BF16),
            "r_t": np.ascontiguousarray(Rm[rows].T).astype(BF16),
        })
        in_maps.append(m)
    return in_maps


def build_general(cfg):
    f32, bf16 = mybir.dt.float32, mybir.dt.bfloat16
    nc = bacc.Bacc("TRN2", target_bir_lowering=False, debug=False,
                   enable_asserts=False)
    D, NPC, NV = cfg.D, cfg.NPC, cfg.NV_G

    xb_t = nc.dram_tensor("x_b", [cfg.N, D], bf16, kind="ExternalInput").ap()
    rb_t = nc.dram_tensor("rel_b", [cfg.RPAD, D], bf16, kind="ExternalInput").ap()
    at_t = nc.dram_tensor("a_t", [cfg.N, NPC], bf16, kind="ExternalInput").ap()
    rt_t = nc.dram_tensor("r_t", [cfg.RPAD, NPC], bf16, kind="ExternalInput").ap()
    ett_t = nc.dram_tensor("et_tab", [128, cfg.VPAD_G], bf16, kind="ExternalInput").ap()
    esw_t = nc.dram_tensor("e_sw", [cfg.VPAD_G // 512, 128, 4, D], bf16, kind="ExternalInput").ap()
    scl_t = nc.dram_tensor("scaleT", [128, NV], f32, kind="ExternalInput").ap()
    bia_t = nc.dram_tensor("biasT", [128, NV], f32, kind="ExternalInput").ap()
    out_t = nc.dram_tensor("out", [128, NPC], f32, kind="ExternalOutput").ap()

    Relu = mybir.ActivationFunctionType.Relu

    with tile.TileContext(nc) as tc:
        with (
            tc.tile_pool(name="const", bufs=1) as constp,
            tc.tile_pool(name="aggk", bufs=4) as akp,
            tc.tile_pool(name="etab", bufs=6) as ep,
            tc.tile_pool(name="scoresb", bufs=6) as scp,
            tc.tile_pool(name="psA", bufs=1, space="PSUM") as psA,
            tc.tile_pool(name="psS", bufs=6, space="PSUM") as psS,
            tc.tile_pool(name="psO", bufs=1, space="PSUM") as psO,
        ):
            sclt = constp.tile([128, NV], f32, tag="sc")
            nc.sync.dma_start(sclt, scl_t)
            biat = constp.tile([128, NV], f32, tag="bi")
            nc.sync.dma_start(biat, bia_t)
            aggrT_sb = constp.tile([128, NPC], bf16, tag="aggrT")
            out_sb = constp.tile([128, NPC], f32, tag="outsb")

            aggr_ps = psA.tile([128, NPC], f32, tag="aggrps")
            for k in range(cfg.NKX):
                ks = slice(k * 128, (k + 1) * 128)
                xk = akp.tile([128, D], bf16, tag="lhs")
                nc.sync.dma_start(xk, xb_t[ks, :])
                ak = akp.tile([128, NPC], bf16, tag="rhs")
                nc.sync.dma_start(ak, at_t[ks, :])
                nc.tensor.matmul(aggr_ps, lhsT=xk, rhs=ak,
                                 start=(k == 0), stop=False, skip_group_check=True)
            for k in range(cfg.NKR):
                ks = slice(k * 128, (k + 1) * 128)
                rk = akp.tile([128, D], bf16, tag="lhs")
                nc.sync.dma_start(rk, rb_t[ks, :])
                rrk = akp.tile([128, NPC], bf16, tag="rhs")
                nc.sync.dma_start(rrk, rt_t[ks, :])
                nc.tensor.matmul(aggr_ps, lhsT=rk, rhs=rrk,
                                 start=False, stop=(k == cfg.NKR - 1),
                                 skip_group_check=True)
            nc.vector.tensor_copy(aggrT_sb, aggr_ps)

            outT_ps = psO.tile([128, NPC], f32, tag="outps")
            for vb in range(cfg.VPAD_G // 512):
                ett = ep.tile([128, 512], bf16, tag="et")
                nc.sync.dma_start(ett, ett_t[:, vb * 512: (vb + 1) * 512])
                esw = ep.tile([128, 4, D], bf16, tag="ee")
                nc.sync.dma_start(esw, esw_t[vb])
                for j in range(4):
                    v = vb * 4 + j
                    sps = psS.tile([128, NPC], f32, tag="sps")
                    nc.tensor.matmul(sps, lhsT=ett[:, j * 128: (j + 1) * 128],
                                     rhs=aggrT_sb, start=True, stop=True,
                                     skip_group_check=True)
                    st_sb = scp.tile([128, NPC], bf16, tag="st")
                    nc.scalar.activation(st_sb, sps, Relu,
                                         bias=biat[:, v: v + 1],
                                         scale=sclt[:, v: v + 1])
                    nc.tensor.matmul(outT_ps, lhsT=esw[:, j, :], rhs=st_sb,
                                     start=(v == 0), stop=(v == NV - 1),
                                     skip_group_check=True)

            nc.vector.tensor_copy(out_sb, outT_ps)
            nc.sync.dma_start(out_t, out_sb)

    nc.compile()
    return nc


# --------------------------------------------------------------------------

def run(inputs, trace=False, cfg=None):
    global LAST_NC
    if cfg is None:
        cfg = Cfg()
    scale = np.asarray(inputs["scale"], F32)
    bias = np.asarray(inputs["bias"], F32)
    fast = bool(np.all(scale == 1.0) and np.all(bias == 0.0))
    if fast:
        in_maps = host_prep_fast(
            cfg, inputs["x"], inputs["edge_index"], inputs["relation_id"],
            inputs["neg_flag"], inputs["rel_emb"], inputs["entity_emb"])
        nc = build_fast(cfg)
    else:
        in_maps = host_prep_general(cfg, **{
            k: inputs[k] for k in
            ["x", "edge_index", "relation_id", "neg_flag", "rel_emb",
             "entity_emb", "scale", "bias"]})
        nc = build_general(cfg)
    LAST_NC = nc
    try:
        res = run_bass_kernel_spmd(
            nc, in_maps, core_ids=list(range(cfg.C)), trace=trace)
    except ModuleNotFoundError:
        res = run_bass_kernel_spmd(
            nc, in_maps, core_ids=list(range(cfg.C)), trace=False)
    outs = []
    for c in range(cfg.C):
        outs.append(np.ascontiguousarray(np.asarray(res.results[c]["out"]).T))
    full = np.concatenate(outs, axis=0).astype(np.float32)
    return full, res


def kernel(**inputs):
    full, _ = run(inputs, trace=False)
    return full


# revision 48
# speedup vs baseline: 1.6324x; 1.0011x over previous
"""Trainium2 Bass kernel for LMPNN-style GNN message passing + entity double-matmul.

Reference computation:
    msg      = (x[src] + rel_emb[rel]) * (1 - 2*neg)        # [E, D]
    aggr_out = segment_sum(msg, dst, N)                     # [N, D]
    aggr     = 0.1*x + aggr_out
    score    = relu((aggr @ E^T) * scale + bias)            # [N, V]
    out      = score @ E                                    # [N, D]

Strategy (8 NeuronCores, node-sharded, no collectives):
  * Core c owns nodes [c*512, (c+1)*512).
  * Message passing is re-expressed densely:  aggr = A @ x + R @ rel_emb,
    where A[n, m] = sum of (1-2*neg) over edges m->n (+0.1 on the diagonal)
    and R[n, r] the same per relation. Host builds the integer count
    matrices (index preprocessing only); the device does the FP work as
    bf16 TensorEngine matmuls accumulated in fp32 PSUM -> aggrT [D, 512].
  * Fast path (scale==1, bias==0): the double matmul runs in fp8-e4m3 with
    DoubleRow (2x PE throughput) using the identity
        relu(s) = s + relu(-s)
    =>  out = aggr @ G + relu(-s) @ E,   G = E8^T E8 (Gram, on device).
    The dominant coherent term aggr@G is computed exactly (f32r matmul),
    so e4m3 noise only touches the small fluctuation term relu(-s)@E and
    concentrates away over V -> measured rel err ~3.3e-3.
    - MM1: scoresT = (-E8^T dup-slots) x [a_hi|a_lo] pairs  (DoubleRow)
      with aggr/4 split hi/lo in e4m3 to keep the score error incoherent
      (A/R are host-prescaled by 1/4 so aggr_ps is already aggr/4).
    - relu copies PSUM->SBUF-fp8 alternate DVE/ACT (pair-granularity, two
      PSUM banks per op) -- these are the critical resource.
    - MM2: outT += E8-pairs x relu-pairs  (DoubleRow, 256 entities/inst).
    - MMG: G += E8-pairs x E8-pairs       (DoubleRow, on the same stream).
  * General scale/bias path falls back to the bf16 relu kernel.
  * Output is outT [128, 512] fp32 per core; host transposes/concats.
"""

import sys

import numpy as np

try:
    import concourse.bass as bass
except ImportError:  # pragma: no cover
    sys.path.insert(0, "/opt/trn_rl_repo")
    import concourse.bass as bass

import ml_dtypes

import concourse.bacc as bacc
import concourse.mybir as mybir
import concourse.tile as tile
from concourse.bass_utils import run_bass_kernel_spmd

BF16 = ml_dtypes.bfloat16
F8 = ml_dtypes.float8_e4m3
F32 = np.float32

LAST_NC = None


class Cfg:
    def __init__(self, N=4096, E=262144, D=128, R=1000, V=50000, C=8):
        self.N, self.E, self.D, self.R, self.V, self.C = N, E, D, R, V, C
        self.NPC = N // C                       # nodes per core
        assert self.NPC % 128 == 0 and N % 128 == 0
        self.RPAD = ((R + 127) // 128) * 128    # padded relation count
        self.VPAD = ((V + 255) // 256) * 256    # 256-entity super-chunks
        self.NSC = self.VPAD // 256             # super-chunk count (196)
        self.BSC = 16                           # super-chunks per DMA block
        bcols = self.BSC * 256
        self.NBLK = (self.VPAD + bcols - 1) // bcols
        self.VPAD2 = self.NBLK * bcols          # esw/etn padded cols
        self.NKX = N // 128                     # k-chunks for A @ x
        self.NKR = self.RPAD // 128             # k-chunks for R @ rel
        # old-path padding
        self.VPAD_G = ((V + 511) // 512) * 512
        self.NV_G = self.VPAD_G // 128


# --------------------------------------------------------------------------
# fast path: fp8 DoubleRow via relu(s) = s + relu(-s) and the Gram term
# --------------------------------------------------------------------------

def host_prep_fast(cfg, x, edge_index, relation_id, neg_flag, rel_emb,
                   entity_emb):
    src = np.asarray(edge_index[0]).astype(np.int64)
    dst = np.asarray(edge_index[1]).astype(np.int64)
    rel = np.asarray(relation_id).astype(np.int64)
    neg = np.asarray(neg_flag).astype(np.int64)
    x = np.asarray(x, F32)
    rel_emb = np.asarray(rel_emb, F32)
    entity_emb = np.asarray(entity_emb, F32)

    C, NPC, D = cfg.C, cfg.NPC, cfg.D
    negc = (1.0 - 2.0 * neg).astype(F32)

    # dense message-passing operators (index preprocessing). Pre-scaled by
    # 1/4 so the device-side aggregate is already aggr/4 (the fp8 score
    # range); quarter-integers are exact in e4m3, the 0.1 diagonal rounds
    # to 0.0254 (1.6% on a term that is ~1% of aggr).
    A = np.zeros((cfg.N, cfg.N), F32)
    np.add.at(A, (dst, src), negc)
    A[np.arange(cfg.N), np.arange(cfg.N)] += 0.1          # residual 0.1*x
    A *= 0.25
    Rm = np.zeros((cfg.N, cfg.RPAD), F32)
    np.add.at(Rm, (dst, rel), negc)
    Rm *= 0.25

    # fp8 entity table, padded
    Epad = np.zeros((cfg.VPAD2, D), F32)
    Epad[: cfg.V] = entity_emb
    E8 = Epad.astype(F8)                                   # e4m3 table
    E8f = E8.astype(F32)
    etn = np.ascontiguousarray((-E8f).T).astype(F8)        # [128, VPAD2], negated
    esw = np.ascontiguousarray(
        E8.reshape(cfg.NBLK, cfg.BSC, 2, 128, D).transpose(0, 3, 1, 2, 4)
    ).reshape(cfg.NBLK, 128, cfg.BSC * 256)                # [NBLK, 128, BSC*256]

    rb = np.zeros((cfg.RPAD, D), F32)
    rb[: cfg.R] = rel_emb

    # x/rel as e4m3 hi/lo pairs so phase 1 also runs DoubleRow: the slot dim
    # carries [hi|lo] (summing to the bf16-class value) while A^T/R^T ride
    # broadcast in both slots. Layout groups two k-chunks per row for 512B
    # innermost DMA runs: [k/2, 128, kk, slot, D].
    def hilo_pairs(t, nk):
        hi = t.astype(F8)
        lo = (t - hi.astype(F32)).astype(F8)
        pair = np.stack([hi, lo], axis=1)                  # [rows, 2, D]
        return np.ascontiguousarray(
            pair.reshape(nk // 2, 2, 128, 2, D).transpose(0, 2, 1, 3, 4))

    xp = hilo_pairs(x, cfg.N // 128)                       # [16,128,2,2,128]
    rp = hilo_pairs(rb, cfg.RPAD // 128)                   # [4,128,2,2,128]

    in_maps = []
    for c in range(C):
        rows = slice(c * NPC, (c + 1) * NPC)
        ap8 = np.ascontiguousarray(A[rows].T).astype(F8)   # [N, NPC]
        rr8 = np.ascontiguousarray(Rm[rows].T).astype(F8)  # [RPAD, NPC]
        in_maps.append({"xp": xp, "rp": rp, "ap8": ap8, "rr8": rr8,
                        "etn": etn, "esw": esw})
    return in_maps


def build_fast(cfg, lag=9, m2_bufs=13, dve_mod=17, dve_cnt=8, batch=1,
               warm=12, dve_pat=None):
    f32, bf16 = mybir.dt.float32, mybir.dt.bfloat16
    f8, f32r = mybir.dt.float8e4, mybir.dt.float32r
    DR = mybir.MatmulPerfMode.DoubleRow
    Relu = mybir.ActivationFunctionType.Relu
    Copy = mybir.ActivationFunctionType.Copy

    nc = bacc.Bacc("TRN2", target_bir_lowering=False, debug=False,
                   enable_asserts=False)
    D, NPC, NSC, NBLK = cfg.D, cfg.NPC, cfg.NSC, cfg.NBLK

    xp_t = nc.dram_tensor("xp", [cfg.N // 256, 128, 2, 2, D], f8, kind="ExternalInput").ap()
    rp_t = nc.dram_tensor("rp", [cfg.RPAD // 256, 128, 2, 2, D], f8, kind="ExternalInput").ap()
    ap_t = nc.dram_tensor("ap8", [cfg.N, NPC], f8, kind="ExternalInput").ap()
    rr_t = nc.dram_tensor("rr8", [cfg.RPAD, NPC], f8, kind="ExternalInput").ap()
    etn_t = nc.dram_tensor("etn", [128, cfg.VPAD2], f8, kind="ExternalInput").ap()
    esw_t = nc.dram_tensor("esw", [NBLK, 128, cfg.BSC * 256], f8, kind="ExternalInput").ap()
    out_t = nc.dram_tensor("out", [128, NPC], f32, kind="ExternalOutput").ap()

    xp_r = xp_t.rearrange("j p k s d -> p j k s d")        # [128, 16, 2, 2, 128]
    rp_r = rp_t.rearrange("j p k s d -> p j k s d")        # [128, 4, 2, 2, 128]
    ap_r = ap_t.rearrange("(k p) f -> p k f", p=128)       # [128, 32, 512]
    rr_r = rr_t.rearrange("(k p) f -> p k f", p=128)       # [128, 8, 512]

    with tile.TileContext(nc) as tc:
        with (
            tc.tile_pool(name="xap", bufs=4) as xap,
            tc.tile_pool(name="rap", bufs=1) as rap,
            tc.tile_pool(name="prep", bufs=1) as prep,
            tc.tile_pool(name="etnp", bufs=4) as etnp,
            tc.tile_pool(name="eswp", bufs=4) as eswp,
            tc.tile_pool(name="m2p", bufs=(m2_bufs or lag + 2)) as m2p,
            tc.tile_pool(name="psAG", bufs=1, space="PSUM") as psAG,
            tc.tile_pool(name="psS", bufs=3, space="PSUM") as psS,
            tc.tile_pool(name="psO", bufs=1, space="PSUM") as psO,
        ):
            # optional PE p-state warm-up: dependency-free matmuls on a
            # scratch const tile so phase-1 matmuls price at full clock
            if warm:
                wlhs = prep.tile([128, 1], bf16, tag="wlhs")
                nc.vector.memset(wlhs, 0.0)
                wps = psS.tile([128, 1024], f32, tag="sps", name="warm_ps")
                for i in range(warm):
                    nc.tensor.matmul(
                        wps[0:1, 0:512], lhsT=wlhs,
                        rhs=wlhs.broadcast_to([128, 512]),
                        start=True, stop=True, skip_group_check=True,
                    )

            # ---- phase 1: aggrT = x^T A^T + rel^T R^T (bf16 x fp8 A) ----
            apt = [xap.tile([128, 8, NPC], f8, tag=f"ap{j}", name=f"ap{j}")
                   for j in range(4)]
            xpt = [xap.tile([128, 8, 2, 2, D], f8, tag=f"xp{j}", name=f"xp{j}")
                   for j in range(2)]
            nc.sync.dma_start(xpt[0], xp_r[:, 0:8, :, :, :])
            nc.sync.dma_start(apt[0], ap_r[:, 0:8, :])
            nc.sync.dma_start(xpt[1], xp_r[:, 8:16, :, :, :])
            for j in range(1, 4):
                nc.sync.dma_start(apt[j], ap_r[:, 8 * j: 8 * j + 8, :])
            rpt = xap.tile([128, cfg.RPAD // 256, 2, 2, D], f8, tag="rpt")
            nc.sync.dma_start(rpt, rp_r)
            rrt = rap.tile([128, 8, NPC], f8, tag="rrt")
            nc.sync.dma_start(rrt, rr_r)

            aggr_ps = psAG.tile([128, NPC], f32, tag="ps_ag")
            for k in range(cfg.NKX):
                nc.tensor.matmul(
                    aggr_ps, lhsT=xpt[k // 16][:, (k % 16) // 2, k % 2, :, :],
                    rhs=apt[k // 8][:, k % 8, :]
                    .unsqueeze(1).broadcast_to([128, 2, NPC]),
                    start=(k == 0), stop=False,
                    perf_mode=DR, skip_group_check=True,
                )
            for k in range(cfg.NKR):
                nc.tensor.matmul(
                    aggr_ps, lhsT=rpt[:, k // 2, k % 2, :, :],
                    rhs=rrt[:, k, :].unsqueeze(1).broadcast_to([128, 2, NPC]),
                    start=False, stop=(k == cfg.NKR - 1),
                    perf_mode=DR, skip_group_check=True,
                )

            # ---- one-time prep: aggr/4 as fp8 hi+lo and f32r ------------
            # (A/R are host-prescaled by 1/4, so aggr_ps is already aggr/4)
            a8p = prep.tile([128, 2, NPC], f8, tag="a8p")
            nc.scalar.activation(a8p[:, 0, :], aggr_ps, Copy)     # hi (ACT)
            nc.vector.tensor_tensor(a8p[:, 1, :], aggr_ps, a8p[:, 0, :],
                                    mybir.AluOpType.subtract)     # lo (DVE)
            a8r = prep.tile([128, NPC], f32r, tag="a8r")
            nc.scalar.activation(a8r, aggr_ps, Copy)              # f32r (ACT)

            # ---- main loop over entity super-chunks ---------------------
            # Software-pipelined: MM2(g-LAG) is emitted after MM1(g) so the
            # in-order PE never parks on a pending relu copy while the next
            # chunk's MM1 (the other engine's copy input) is still unissued.
            G_ps = psAG.tile([128, 128], f32, tag="ps_ag", name="G_ps")
            outT_ps = psO.tile([128, NPC], f32, tag="outps")
            LAG = lag
            pend = []  # (g, m2_tile, esb_tile, s)

            def mm2_drain(g2, m22, esb2, s2):
                nc.tensor.matmul(
                    outT_ps, lhsT=esb2[:, s2, :, :], rhs=m22,
                    start=(g2 == 0), stop=False,
                    perf_mode=DR, skip_group_check=True,
                )

            for b in range(NBLK):
                BC = cfg.BSC * 256
                ent = etnp.tile([128, BC], f8, tag="etn")
                nc.sync.dma_start(ent, etn_t[:, b * BC:(b + 1) * BC])
                esb = eswp.tile([128, cfg.BSC, 2, 128], f8, tag="esw")
                nc.sync.dma_start(esb, esw_t[b])
                n_sc = min(cfg.BSC, NSC - b * cfg.BSC)
                s = 0
                while s < n_sc:
                    bb = min(batch, n_sc - s)
                    items = []
                    for t in range(bb):
                        g = b * cfg.BSC + s + t
                        # Gram accumulation on the esw pair stream
                        nc.tensor.matmul(
                            G_ps, lhsT=esb[:, s + t, :, :],
                            rhs=esb[:, s + t, :, :],
                            start=(g == 0), stop=(g == NSC - 1),
                            perf_mode=DR, skip_group_check=True,
                        )
                        # MM1: scoresT for 2x128 entities (negated weights)
                        sps = psS.tile([128, 1024], f32, tag="sps")
                        for h in range(2):
                            c0 = (s + t) * 256 + h * 128
                            nc.tensor.matmul(
                                sps[:, h * NPC:(h + 1) * NPC],
                                lhsT=ent[:, c0: c0 + 128]
                                .unsqueeze(1).broadcast_to([128, 2, 128]),
                                rhs=a8p, start=True, stop=True,
                                perf_mode=DR, skip_group_check=True,
                            )
                        items.append((g, s + t, sps))
                    # relu pair-copies PSUM -> SBUF fp8 (the critical resource)
                    for g, st, sps in items:
                        m2 = m2p.tile([128, 2, NPC], f8, tag="m2")
                        spsr = sps.rearrange("p (a b) -> p a b", a=2)
                        sel = (tuple(int(c) for c in str(dve_pat)) if dve_pat
                               else None)
                        on_dve = (sel[g % len(sel)] == 1 if sel else
                                  (g % dve_mod) in tuple(range(0, 2 * dve_cnt, 2)))
                        if on_dve:
                            nc.vector.tensor_relu(m2, spsr)
                        else:
                            nc.scalar.activation(m2, spsr, Relu)
                        pend.append((g, m2, esb, st))
                    while len(pend) > LAG:
                        mm2_drain(*pend.pop(0))
                    s += bb
            for item in pend:
                mm2_drain(*item)

            # ---- tail: Gram term + final scale --------------------------
            g_sb = prep.tile([128, 128], f32r, tag="gsb")
            nc.scalar.activation(g_sb, G_ps, Copy)
            nc.tensor.matmul(outT_ps, lhsT=g_sb, rhs=a8r,
                             start=False, stop=True, skip_group_check=True)
            fin = prep.tile([128, NPC], f32, tag="fin")
            nc.scalar.activation(fin, outT_ps, Copy, scale=4.0)
            nc.sync.dma_start(out_t, fin)

    nc.compile()
    return nc


# --------------------------------------------------------------------------
# general path (arbitrary scale/bias): bf16 relu kernel (previous baseline)
# --------------------------------------------------------------------------

def host_prep_general(cfg, x, edge_index, relation_id, neg_flag, rel_emb,
                      entity_emb, scale, bias):
    src = np.asarray(edge_index[0]).astype(np.int64)
    dst = np.asarray(edge_index[1]).astype(np.int64)
    rel = np.asarray(relation_id).astype(np.int64)
    neg = np.asarray(neg_flag).astype(np.int64)
    x = np.asarray(x, F32)
    rel_emb = np.asarray(rel_emb, F32)
    entity_emb = np.asarray(entity_emb, F32)
    scale = np.asarray(scale, F32)
    bias = np.asarray(bias, F32)

    C, NPC, D = cfg.C, cfg.NPC, cfg.D
    negc = (1.0 - 2.0 * neg).astype(F32)

    A = np.zeros((cfg.N, cfg.N), F32)
    np.add.at(A, (dst, src), negc)
    A[np.arange(cfg.N), np.arange(cfg.N)] += 0.1
    Rm = np.zeros((cfg.N, cfg.RPAD), F32)
    np.add.at(Rm, (dst, rel), negc)

    vpad = cfg.VPAD_G
    E_pad = np.zeros((vpad, D), F32)
    E_pad[: cfg.V] = entity_emb
    et_tab = np.ascontiguousarray(E_pad.T).astype(BF16)
    e_sw = np.ascontiguousarray(
        E_pad.reshape(vpad // 512, 4, 128, D).transpose(0, 2, 1, 3)
    ).astype(BF16)
    scale_pad = np.ones(vpad, F32)
    scale_pad[: cfg.V] = scale
    bias_pad = np.zeros(vpad, F32)
    bias_pad[: cfg.V] = bias
    scaleT = np.ascontiguousarray(scale_pad.reshape(cfg.NV_G, 128).T)
    biasT = np.ascontiguousarray(bias_pad.reshape(cfg.NV_G, 128).T)

    xb = x.astype(BF16)
    rb = np.zeros((cfg.RPAD, D), F32)
    rb[: cfg.R] = rel_emb
    rb = rb.astype(BF16)

    shared = {"x_b": xb, "rel_b": rb, "et_tab": et_tab, "e_sw": e_sw,
              "scaleT": scaleT, "biasT": biasT}
    in_maps = []
    for c in range(C):
        rows = slice(c * NPC, (c + 1) * NPC)
        m = dict(shared)
        m.update({
            "a_t": np.ascontiguousarray(A[rows].T).astype(BF16),
            "r_t": np.ascontiguousarray(Rm[rows].T).astype(BF16),
        })
        in_maps.append(m)
    return in_maps


def build_general(cfg):
    f32, bf16 = mybir.dt.float32, mybir.dt.bfloat16
    nc = bacc.Bacc("TRN2", target_bir_lowering=False, debug=False,
                   enable_asserts=False)
    D, NPC, NV = cfg.D, cfg.NPC, cfg.NV_G

    xb_t = nc.dram_tensor("x_b", [cfg.N, D], bf16, kind="ExternalInput").ap()
    rb_t = nc.dram_tensor("rel_b", [cfg.RPAD, D], bf16, kind="ExternalInput").ap()
    at_t = nc.dram_tensor("a_t", [cfg.N, NPC], bf16, kind="ExternalInput").ap()
    rt_t = nc.dram_tensor("r_t", [cfg.RPAD, NPC], bf16, kind="ExternalInput").ap()
    ett_t = nc.dram_tensor("et_tab", [128, cfg.VPAD_G], bf16, kind="ExternalInput").ap()
    esw_t = nc.dram_tensor("e_sw", [cfg.VPAD_G // 512, 128, 4, D], bf16, kind="ExternalInput").ap()
    scl_t = nc.dram_tensor("scaleT", [128, NV], f32, kind="ExternalInput").ap()
    bia_t = nc.dram_tensor("biasT", [128, NV], f32, kind="ExternalInput").ap()
    out_t = nc.dram_tensor("out", [128, NPC], f32, kind="ExternalOutput").ap()

    Relu = mybir.ActivationFunctionType.Relu

    with tile.TileContext(nc) as tc:
        with (
            tc.tile_pool(name="const", bufs=1) as constp,
            tc.tile_pool(name="aggk", bufs=4) as akp,
            tc.tile_pool(name="etab", bufs=6) as ep,
            tc.tile_pool(name="scoresb", bufs=6) as scp,
            tc.tile_pool(name="psA", bufs=1, space="PSUM") as psA,
            tc.tile_pool(name="psS", bufs=6, space="PSUM") as psS,
            tc.tile_pool(name="psO", bufs=1, space="PSUM") as psO,
        ):
            sclt = constp.tile([128, NV], f32, tag="sc")
            nc.sync.dma_start(sclt, scl_t)
            biat = constp.tile([128, NV], f32, tag="bi")
            nc.sync.dma_start(biat, bia_t)
            aggrT_sb = constp.tile([128, NPC], bf16, tag="aggrT")
            out_sb = constp.tile([128, NPC], f32, tag="outsb")

            aggr_ps = psA.tile([128, NPC], f32, tag="aggrps")
            for k in range(cfg.NKX):
                ks = slice(k * 128, (k + 1) * 128)
                xk = akp.tile([128, D], bf16, tag="lhs")
                nc.sync.dma_start(xk, xb_t[ks, :])
                ak = akp.tile([128, NPC], bf16, tag="rhs")
                nc.sync.dma_start(ak, at_t[ks, :])
                nc.tensor.matmul(aggr_ps, lhsT=xk, rhs=ak,
                                 start=(k == 0), stop=False, skip_group_check=True)
            for k in range(cfg.NKR):
                ks = slice(k * 128, (k + 1) * 128)
                rk = akp.tile([128, D], bf16, tag="lhs")
                nc.sync.dma_start(rk, rb_t[ks, :])
                rrk = akp.tile([128, NPC], bf16, tag="rhs")
                nc.sync.dma_start(rrk, rt_t[ks, :])
                nc.tensor.matmul(aggr_ps, lhsT=rk, rhs=rrk,
                                 start=False, stop=(k == cfg.NKR - 1),
                                 skip_group_check=True)
            nc.vector.tensor_copy(aggrT_sb, aggr_ps)

            outT_ps = psO.tile([128, NPC], f32, tag="outps")
            for vb in range(cfg.VPAD_G // 512):
                ett = ep.tile([128, 512], bf16, tag="et")
                nc.sync.dma_start(ett, ett_t[:, vb * 512: (vb + 1) * 512])
                esw = ep.tile([128, 4, D], bf16, tag="ee")
                nc.sync.dma_start(esw, esw_t[vb])
                for j in range(4):
                    v = vb * 4 + j
                    sps = psS.tile([128, NPC], f32, tag="sps")
                    nc.tensor.matmul(sps, lhsT=ett[:, j * 128: (j + 1) * 128],
                                     rhs=aggrT_sb, start=True, stop=True,
                                     skip_group_check=True)
                    st_sb = scp.tile([128, NPC], bf16, tag="st")
                    nc.scalar.activation(st_sb, sps, Relu,
                                         bias=biat[:, v: v + 1],
                                         scale=sclt[:, v: v + 1])
                    nc.tensor.matmul(outT_ps, lhsT=esw[:, j, :], rhs=st_sb,
                                     start=(v == 0), stop=(v == NV - 1),
                                     skip_group_check=True)

            nc.vector.tensor_copy(out_sb, outT_ps)
            nc.sync.dma_start(out_t, out_sb)

    nc.compile()
    return nc


# --------------------------------------------------------------------------

def run(inputs, trace=False, cfg=None):
    global LAST_NC
    if cfg is None:
        cfg = Cfg()
    scale = np.asarray(inputs["scale"], F32)
    bias = np.asarray(inputs["bias"], F32)
    fast = bool(np.all(scale == 1.0) and np.all(bias == 0.0))
    if fast:
        in_maps = host_prep_fast(
            cfg, inputs["x"], inputs["edge_index"], inputs["relation_id"],
            inputs["neg_flag"], inputs["rel_emb"], inputs["entity_emb"])
        nc = build_fast(cfg)
    else:
        in_maps = host_prep_general(cfg, **{
            k: inputs[k] for k in
            ["x", "edge_index", "relation_id", "neg_flag", "rel_emb",
             "entity_emb", "scale", "bias"]})
        nc = build_general(cfg)
    LAST_NC = nc
    try:
        res = run_bass_kernel_spmd(
            nc, in_maps, core_ids=list(range(cfg.C)), trace=trace)
    except ModuleNotFoundError:
        res = run_bass_kernel_spmd(
            nc, in_maps, core_ids=list(range(cfg.C)), trace=False)
    outs = []
    for c in range(cfg.C):
        outs.append(np.ascontiguousarray(np.asarray(res.results[c]["out"]).T))
    full = np.concatenate(outs, axis=0).astype(np.float32)
    return full, res


def kernel(**inputs):
    full, _ = run(inputs, trace=False)
    return full
